# revision 1
# baseline (speedup 1.0000x reference)
"""Trainium2 Bass kernel for nn_MultiHeadAttention_69930657513858.

Single-token (decode) multi-head attention, B=8, E=4096, H=32 heads of
D=128, with a KV cache that is identically ones (length L=4095).

Because the cache is all-ones, attention collapses to a closed form:
  scores = [s0]*L ++ [s1],  s0 = sum_d(q)/sqrt(D), s1 = (q.k)/sqrt(D)
  softmax => p_last = sigmoid(s1 - s0 - ln(L)); cache mass = 1 - p_last
  o = (1 - p_last)*ones + p_last*v = 1 + p_last*(v - 1)
so the kernel is four GEMMs (q,k,v projections + out-proj) plus O(B*H)
scalar work.  Sharding: tensor-parallel over heads, 4 heads per core
(Wq/Wk/Wv row-sliced, Wo column-sliced); partial out-proj results are
summed on the host (the "all-reduce").

The PE contracts over the partition dim, so each GEMM needs the weight's
in_features on partitions; weight slices are pre-transposed on the host
while sharding (same class of host work as the slicing itself).

MODE selects the matmul numerics:
  fp32  - exact fp32 matmuls (4 PE cycles/row)
  f32r  - fp32 storage, replicated-mode matmul (1 cycle/row, reduced
          multiply precision)
  split - weights & x shipped as bf16 hi+lo pairs; x@W = xhi@Whi +
          xlo@Whi + xhi@Wlo accumulated in fp32 PSUM (2 cycles/row,
          ~1e-5 rel err, full fp32 DMA traffic)
  bf16  - plain bf16 weights/x (1 cycle/row, half DMA traffic,
          ~2e-3 rel err)
"""

import math
import os

import numpy as np

B = 8
E = 4096
H = 32
D = 128
L = 4095
N_CORES = 8
HPC = H // N_CORES  # heads per core = 4
F = HPC * D  # per-core head width = 512
ET = E // 128  # e tiles = 32
SCALE = 1.0 / math.sqrt(D)
BIAS = -math.log(L)

MODE = os.environ.get("MHA_MODE", "split")

# DMA chunking for the q/k/v weight streams: e-tiles per DMA (1 MiB)
EC = 4
NCHUNK = ET // EC

_CACHE = {}


def _build_program(mode):
    import concourse.mybir as mybir
    import concourse.tile as tile
    from concourse import bacc
    from concourse.masks import make_identity

    fp32 = mybir.dt.float32
    bf16 = mybir.dt.bfloat16
    split = mode == "split"
    wdt = bf16 if mode in ("split", "bf16") else fp32
    if mode == "f32r":
        wdt = mybir.dt.float32r
    # dtype used on the ovec->zT transpose path (tsrc/ident/psum)
    tdt = bf16 if mode in ("split", "bf16") else fp32
    # stationary x holds hi (rows 0-7) and lo (rows 32-39) halves in
    # split mode; partition-base APs must start at multiples of 32
    XW = 40 if split else B

    nc = bacc.Bacc("TRN2", target_bir_lowering=False)

    # split: q/k/v ship chunk-interleaved hi/lo rows ([2E, F]); wo ships
    # stacked halves ([2, F, E]) whose strides merge naturally
    qsh = [2 * E, F] if split else [E, F]
    osh = [2, F, E] if split else [F, E]
    wq = nc.dram_tensor("wq_t", qsh, wdt, kind="ExternalInput").ap()
    wk = nc.dram_tensor("wk_t", qsh, wdt, kind="ExternalInput").ap()
    wv = nc.dram_tensor("wv_t", qsh, wdt, kind="ExternalInput").ap()
    wo = nc.dram_tensor("wo_t", osh, wdt, kind="ExternalInput").ap()
    xw_in = 2 * B if split else B
    xt = nc.dram_tensor("xt", [E, xw_in], wdt, kind="ExternalInput").ap()
    out = nc.dram_tensor("out_p", [B, E], fp32, kind="ExternalOutput").ap()

    if split:
        def wr(ap):  # [2E, F] chunk-interleaved -> [128, NCHUNK, 2, EC, F]
            return ap.rearrange("(c s j p) f -> p c s j f", p=128, s=2, j=EC)

        wo_r = wo.rearrange("s (t p) e -> p s t e", p=128)
    else:
        def wr(ap):  # [E, F] -> [128, ET, F]
            return ap.rearrange("(c p) f -> p c f", p=128)

        wo_r = wo.rearrange("(t p) e -> p t e", p=128)  # [128, HPC, E]
    xt_r = xt.rearrange("(t p) b -> p t b", p=128)  # [128, ET, xw_in]

    with tile.TileContext(nc) as tc:
        with (
            tc.tile_pool(name="const", bufs=1) as const_pool,
            tc.tile_pool(name="wqp", bufs=3) as wq_pool,
            tc.tile_pool(name="wkp", bufs=3) as wk_pool,
            tc.tile_pool(name="wvp", bufs=3) as wv_pool,
            tc.tile_pool(name="wop", bufs=6) as wo_pool,
            tc.tile_pool(name="small", bufs=1) as small_pool,
            tc.tile_pool(name="outp", bufs=3) as out_pool,
            tc.tile_pool(name="ps_qkv", bufs=1, space="PSUM") as ps_qkv,
            tc.tile_pool(name="ps_t", bufs=1, space="PSUM") as ps_t,
            tc.tile_pool(name="ps_o", bufs=4, space="PSUM") as ps_o,
        ):
            ident = const_pool.tile([128, 128], tdt)
            make_identity(nc, ident[:])
            bias_sb = const_pool.tile([128, 1], fp32, tag="bias")
            nc.gpsimd.memset(bias_sb[:], BIAS)

            xin_sb = const_pool.tile([128, ET, xw_in], wdt, tag="xin")
            nc.scalar.dma_start(xin_sb[:], xt_r)
            if split:
                # widen to [xhi | 0 | xlo] (cols 0-7 / 32-39) and build the
                # [xhi | 0] stationary for the lo-weight pass on device
                xt_sb = const_pool.tile([128, ET, XW], wdt, tag="xt40")
                nc.gpsimd.memset(xt_sb[:], 0.0)
                nc.vector.tensor_copy(xt_sb[:, :, :B], xin_sb[:, :, :B])
                nc.vector.tensor_copy(xt_sb[:, :, 32:40], xin_sb[:, :, B:])
                xt0_sb = const_pool.tile([128, ET, XW], wdt, tag="xt0")
                nc.gpsimd.memset(xt0_sb[:], 0.0)
                nc.vector.tensor_copy(xt0_sb[:, :, :B], xin_sb[:, :, :B])
            else:
                xt_sb = xin_sb

            # ---- q/k/v projections ----
            psum_q = ps_qkv.tile([XW, F], fp32, tag="psq")
            psum_k = ps_qkv.tile([XW, F], fp32, tag="psk")
            psum_v = ps_qkv.tile([XW, F], fp32, tag="psv")

            for c in range(NCHUNK):
                sls = slice(c * EC, (c + 1) * EC)
                if split:
                    w_sb = {}
                    for nm, ap, pool in (
                        ("q", wr(wq), wq_pool),
                        ("k", wr(wk), wk_pool),
                        ("v", wr(wv), wv_pool),
                    ):
                        t2 = pool.tile([128, 2, EC, F], wdt, tag="w" + nm)
                        nc.sync.dma_start(t2[:], ap[:, c])
                        w_sb[nm] = t2
                else:
                    w_sb = {}
                    for nm, ap, pool in (
                        ("q", wr(wq), wq_pool),
                        ("k", wr(wk), wk_pool),
                        ("v", wr(wv), wv_pool),
                    ):
                        t1 = pool.tile([128, EC, F], wdt, tag="w" + nm)
                        nc.sync.dma_start(t1[:], ap[:, sls, :])
                        w_sb[nm] = t1
                for j in range(EC):
                    t = c * EC + j
                    first, last = t == 0, t == ET - 1
                    lhs_full = xt_sb[:, t, :]
                    for nm, ps in (("q", psum_q), ("k", psum_k), ("v", psum_v)):
                        if split:
                            nc.tensor.matmul(
                                ps[:], lhs_full, w_sb[nm][:, 0, j, :],
                                start=first, stop=False,
                            )
                            nc.tensor.matmul(
                                ps[:], xt0_sb[:, t, :], w_sb[nm][:, 1, j, :],
                                start=False, stop=last,
                            )
                        else:
                            nc.tensor.matmul(
                                ps[:], lhs_full, w_sb[nm][:, j, :],
                                start=first, stop=last,
                            )

            # ---- combine split halves; closed-form attention ----
            q_sb = small_pool.tile([B, F], fp32, tag="qsb")
            k_sb = small_pool.tile([B, F], fp32, tag="ksb")
            v_sb = small_pool.tile([B, F], fp32, tag="vsb")
            if split:
                tmp = small_pool.tile([B, F], fp32, tag="tmp")
                for ps, dst in ((psum_q, q_sb), (psum_k, k_sb), (psum_v, v_sb)):
                    nc.vector.tensor_copy(tmp[:], ps[32:40, :])
                    nc.vector.tensor_tensor(
                        dst[:], ps[:B, :], tmp[:], mybir.AluOpType.add
                    )
            else:
                nc.vector.tensor_copy(q_sb[:], psum_q[:])
                nc.vector.tensor_copy(k_sb[:], psum_k[:])
                nc.vector.tensor_copy(v_sb[:], psum_v[:])

            s0 = small_pool.tile([B, HPC], fp32, tag="s0")
            s1 = small_pool.tile([B, HPC], fp32, tag="s1")
            qk = small_pool.tile([B, F], fp32, tag="qk")
            tt = small_pool.tile([B, HPC], fp32, tag="tt")
            p = small_pool.tile([B, HPC], fp32, tag="p")
            ovec = small_pool.tile([B, F], fp32, tag="ovec")

            nc.vector.reduce_sum(
                s0[:], q_sb[:].rearrange("b (h d) -> b h d", d=D),
                axis=mybir.AxisListType.X,
            )
            nc.vector.tensor_tensor(
                qk[:], q_sb[:], k_sb[:], mybir.AluOpType.mult
            )
            nc.vector.reduce_sum(
                s1[:], qk[:].rearrange("b (h d) -> b h d", d=D),
                axis=mybir.AxisListType.X,
            )
            nc.vector.tensor_tensor(
                tt[:], s1[:], s0[:], mybir.AluOpType.subtract
            )
            # p = sigmoid((s1 - s0) * (1/sqrt(D)) - ln(L))
            nc.scalar.activation(
                p[:], tt[:], mybir.ActivationFunctionType.Sigmoid,
                bias=bias_sb[:B, :], scale=SCALE,
            )
            vm1 = small_pool.tile([B, F], fp32, tag="vm1")
            nc.vector.tensor_scalar_add(vm1[:], v_sb[:], -1.0)
            for h in range(HPC):
                sl = slice(h * D, (h + 1) * D)
                nc.vector.tensor_scalar(
                    ovec[:, sl], vm1[:, sl],
                    p[:, h : h + 1], 1.0,
                    mybir.AluOpType.mult, mybir.AluOpType.add,
                )

            # ---- transpose ovec -> zT [f, b] (PE transposes) ----
            if split:
                # hi/lo split of ovec: hi rows 0-7, lo rows 32-39
                z2 = small_pool.tile([XW, F], tdt, tag="z2")
                zf = small_pool.tile([B, F], fp32, tag="zf")
                nc.gpsimd.memset(z2[:], 0.0)
                nc.vector.tensor_copy(z2[:B, :], ovec[:])  # cast to bf16
                nc.vector.tensor_copy(zf[:], z2[:B, :])  # back to f32
                nc.vector.tensor_tensor(
                    zf[:], ovec[:], zf[:], mybir.AluOpType.subtract
                )
                nc.vector.tensor_copy(z2[32:40, :], zf[:])  # lo in bf16
                z3 = small_pool.tile([XW, F], tdt, tag="z3")
                nc.gpsimd.memset(z3[:], 0.0)
                nc.vector.tensor_copy(z3[:B, :], z2[:B, :])
                tsrc, tp = z2, XW
            elif mode == "bf16":
                z1 = small_pool.tile([B, F], tdt, tag="z1")
                nc.vector.tensor_copy(z1[:], ovec[:])
                tsrc, tp = z1, B
            else:
                tsrc, tp = ovec, B
            zt_sb = small_pool.tile([128, HPC, XW], wdt, tag="zt")
            for t in range(HPC):
                zt_ps = ps_t.tile([128, tp], tdt, tag="ztps")
                nc.tensor.transpose(
                    zt_ps[:], tsrc[:, t * 128 : (t + 1) * 128], ident[:tp, :tp]
                )
                nc.vector.tensor_copy(zt_sb[:, t, :], zt_ps[:])
            if split:
                zt0_sb = small_pool.tile([128, HPC, XW], wdt, tag="zt0")
                for t in range(HPC):
                    zt_ps = ps_t.tile([128, tp], tdt, tag="ztps")
                    nc.tensor.transpose(
                        zt_ps[:], z3[:, t * 128 : (t + 1) * 128], ident[:tp, :tp]
                    )
                    nc.vector.tensor_copy(zt0_sb[:, t, :], zt_ps[:])

            # ---- out-proj (wo streamed in per-chunk DMAs so the MMs
            # pipeline with the transfer instead of a serial tail) ----
            # 6 x 512-wide chunks, then 4 x 256-wide for a shorter drain
            chunks = [(k * 512, 512) for k in range(6)] + [
                (3072 + k * 256, 256) for k in range(4)
            ]
            NOC = len(chunks)
            o_acc = small_pool.tile([B, E], fp32, tag="oacc")
            for c2, (off, w) in enumerate(chunks):
                sl2 = slice(off, off + w)
                if split:
                    wo_sb = wo_pool.tile([128, 2, HPC, 512], wdt, tag="wo")
                    nc.sync.dma_start(wo_sb[:, :, :, :w], wo_r[:, :, :, sl2])
                else:
                    wo_sb = wo_pool.tile([128, HPC, 512], wdt, tag="wo")
                    nc.sync.dma_start(wo_sb[:, :, :w], wo_r[:, :, sl2])
                psum_o = ps_o.tile([XW, 512], fp32, tag="pso")
                psum_o = psum_o[:, :w]
                for t in range(HPC):
                    if split:
                        nc.tensor.matmul(
                            psum_o[:], zt_sb[:, t, :], wo_sb[:, 0, t, :w],
                            start=(t == 0), stop=False,
                        )
                        nc.tensor.matmul(
                            psum_o[:], zt0_sb[:, t, :], wo_sb[:, 1, t, :w],
                            start=False, stop=(t == HPC - 1),
                        )
                    else:
                        nc.tensor.matmul(
                            psum_o[:], zt_sb[:, t, :], wo_sb[:, t, :w],
                            start=(t == 0), stop=(t == HPC - 1),
                        )
                if split:
                    ol_sb = out_pool.tile([B, 512], fp32, tag="olsb")
                    nc.vector.tensor_copy(ol_sb[:, :w], psum_o[32:40, :])
                    nc.vector.tensor_tensor(
                        o_acc[:, sl2], psum_o[:B, :], ol_sb[:, :w],
                        mybir.AluOpType.add,
                    )
                else:
                    nc.vector.tensor_copy(o_acc[:, sl2], psum_o[:])
                if c2 == 5:
                    # staggered early writes keep all write dispatch (and
                    # its HWDGE setup) off the final-write critical path
                    nc.scalar.dma_start(out[:, :3072], o_acc[:, :3072])
                elif c2 == NOC - 2:
                    nc.scalar.dma_start(out[:, 3072:3840], o_acc[:, 3072:3840])
            nc.sync.dma_start(out[:, 3840:], o_acc[:, 3840:])

    nc.compile()
    return nc


def _get_program(mode=MODE):
    key = "nc_" + mode
    if key not in _CACHE:
        _CACHE[key] = _build_program(mode)
    return _CACHE[key]


def _split_pair(a):
    import ml_dtypes

    hi = a.astype(ml_dtypes.bfloat16)
    lo = (a - hi.astype(np.float32)).astype(ml_dtypes.bfloat16)
    return hi, lo


def _shard_inputs(x, Wq, Wk, Wv, Wo, mode=MODE):
    import ml_dtypes

    xt = np.ascontiguousarray(x.reshape(B, E).T)
    in_maps = []
    if mode == "split":
        xh, xl = _split_pair(xt)
        xt2 = np.concatenate([xh, xl], axis=1)  # [E, 16]
    elif mode == "bf16":
        xt2 = xt.astype(ml_dtypes.bfloat16)
    else:
        xt2 = xt
    for c in range(N_CORES):
        rows = slice(c * F, (c + 1) * F)
        wqt = np.ascontiguousarray(Wq[rows, :].T)
        wkt = np.ascontiguousarray(Wk[rows, :].T)
        wvt = np.ascontiguousarray(Wv[rows, :].T)
        wot = np.ascontiguousarray(Wo[:, rows].T)
        m = {"xt": xt2}
        if mode == "split":
            for nm, w in (("wq", wqt), ("wk", wkt), ("wv", wvt)):
                hi, lo = _split_pair(w)
                hi = hi.reshape(NCHUNK, EC * 128, F)
                lo = lo.reshape(NCHUNK, EC * 128, F)
                m[nm + "_t"] = np.ascontiguousarray(
                    np.stack([hi, lo], axis=1)
                ).reshape(2 * E, F)
            hi, lo = _split_pair(wot)
            m["wo_t"] = np.stack([hi, lo])
        elif mode == "bf16":
            for nm, w in (("wq", wqt), ("wk", wkt), ("wv", wvt), ("wo", wot)):
                m[nm + "_t"] = w.astype(ml_dtypes.bfloat16)
        else:
            m.update(wq_t=wqt, wk_t=wkt, wv_t=wvt, wo_t=wot)
        in_maps.append(m)
    return in_maps


def kernel(x, Wq, Wk, Wv, Wo, _trace=False, **_unused):
    from concourse.bass_utils import run_bass_kernel_spmd

    nc = _get_program()
    in_maps = _shard_inputs(
        np.asarray(x, dtype=np.float32),
        np.asarray(Wq, dtype=np.float32),
        np.asarray(Wk, dtype=np.float32),
        np.asarray(Wv, dtype=np.float32),
        np.asarray(Wo, dtype=np.float32),
    )
    core_ids = list(range(N_CORES))

    def _run(trace):
        return run_bass_kernel_spmd(nc, in_maps, core_ids, trace=trace)

    res = None
    if _trace:
        try:
            res = _run(True)
        except Exception:
            # NTFF profiling hooks unavailable in this environment
            res = None
    if res is None:
        # transient device wedges (NRT_EXEC_UNIT_UNRECOVERABLE) heal after
        # a terminal-side reset; tear down the PJRT client and back off
        # before each retry
        import time as _time

        last = None
        for attempt in range(3):
            try:
                res = _run(False)
                break
            except Exception as e:
                last = e
                try:
                    import jax._src.xla_bridge as _xb

                    _xb._clear_backends()
                except Exception:
                    pass
                _time.sleep(15 * (attempt + 1))
        else:
            raise last
    _CACHE["last_results"] = res
    acc = np.zeros((B, E), np.float32)
    for r in res.results:
        acc += r["out_p"]
    return acc.reshape(B, 1, E)



# revision 6
# speedup vs baseline: 1.9160x; 1.9160x over previous
"""Trainium2 Bass kernel for nn_MultiHeadAttention_69930657513858.

Single-token (decode) multi-head attention, B=8, E=4096, H=32 heads of
D=128, with a KV cache that is identically ones (length L=4095).

Because the cache is all-ones, attention collapses to a closed form:
  scores = [s0]*L ++ [s1],  s0 = sum_d(q)/sqrt(D), s1 = (q.k)/sqrt(D)
  softmax => p_last = sigmoid(s1 - s0 - ln(L)); cache mass = 1 - p_last
  o = (1 - p_last)*ones + p_last*v = 1 + p_last*(v - 1)
so the kernel is four GEMMs (q,k,v projections + out-proj) plus O(B*H)
scalar work, and the output decomposes as
  out = colsum(Wo) + (p*(v-1)) @ Wo^T
where the colsum term dominates (p is mostly tiny).

The kernel is pure weight streaming (~2 FLOPs/byte), so the only lever
is bytes per weight element.  All four weights ship as fp8 e3m4 (4
mantissa bits), scaled into e3m4's +-15.5 range; the colsum term - the
only place where fp8 rounding of Wo would visibly hurt - is shipped as
an exact fp32 vector (16KB) and seeded into the out-proj PSUM via
rank-1 ones-matmuls, so fp8 error only touches the small correction
term (measured rel err ~4e-3 vs the 2e-2 gate).

Matmuls run W-stationary (weight tile [128,128] stationary, x/z [128,8]
moving), so per-matmul PE time is 8 rows and results come out
transposed [e, b]; the host gather untransposes.  Sharding: tensor
parallel over heads, 4 heads per core; partial out-proj results are
summed on the host (the "all-reduce").

Scale bookkeeping (powers of 2, exact in fp32):
  x*2, W*64 in e3m4  =>  q^,k^,v^ = 128*(q,k,v) in PSUM
  p = sigmoid((s1^ - 128*s0^) * SCALE/128^2 - ln L)
  z8 = e3m4(4*p*(v-1)) = (v^/32 - 4) * p
  psum_out = z8 @ (64*Wo) = 256*corr;  colsum ships pre-scaled *256
  out = psum_out / 256
"""

import math

import numpy as np

B = 8
E = 4096
H = 32
D = 128
L = 4095
N_CORES = 8
HPC = H // N_CORES  # heads per core = 4
F = HPC * D  # per-core head width = 512
ET = E // 128  # contraction tiles for q/k/v = 32
FT = HPC  # contraction tiles for out-proj = 4
ECN = E // 128  # output column chunks for out-proj = 32
HB = HPC * B  # (head, batch) pairs per core = 32
SCALE = 1.0 / math.sqrt(D)
BIAS = -math.log(L)

SX = 2.0  # x pre-scale
SW = 64.0  # weight pre-scale
SZ = 4.0  # z pre-scale
SQ = SX * SW  # q/k/v PSUM scale = 128
SO = SZ * SW  # out-proj PSUM scale = 256

WCH = 8  # wo DMA chunks (4 e-chunks = 256KB each)
ECPC = ECN // WCH  # e-chunks per wo DMA chunk = 4

MODE = "fp8"

_CACHE = {}


def _build_program():
    import concourse.mybir as mybir
    import concourse.tile as tile
    from concourse import bacc

    fp32 = mybir.dt.float32
    e3 = mybir.dt.float8e3
    AL = mybir.AluOpType

    nc = bacc.Bacc("TRN2", target_bir_lowering=False)

    # DRAM layouts are partition-major, prepped on the host:
    #   wq8[p,t,f]     = e3m4(64*Wq[cF+f, t*128+p])      (same wk8, wv8)
    #   wo8[p,ec,ft,e] = e3m4(64*Wo[ec*128+e, cF+ft*128+p])
    #   xt8[p,t,b]     = e3m4(2*x[b, t*128+p])
    #   cs[0,ec,e]     = 256*sum_f Wo[ec*128+e, cF+f]    (fp32, exact)
    wq = nc.dram_tensor("wq8", [128, ET, F], e3, kind="ExternalInput").ap()
    wk = nc.dram_tensor("wk8", [128, ET, F], e3, kind="ExternalInput").ap()
    wv = nc.dram_tensor("wv8", [128, ET, F], e3, kind="ExternalInput").ap()
    wo = nc.dram_tensor("wo8", [128, ECN, FT, 128], e3, kind="ExternalInput").ap()
    xt = nc.dram_tensor("xt8", [128, ET, B], e3, kind="ExternalInput").ap()
    cs = nc.dram_tensor("cs", [1, ECN, 128], fp32, kind="ExternalInput").ap()
    out = nc.dram_tensor("out_p", [128, ECN, B], fp32, kind="ExternalOutput").ap()

    with tile.TileContext(nc) as tc:
        with (
            tc.tile_pool(name="wp", bufs=1) as wp,
            tc.tile_pool(name="sp", bufs=1) as sp,
            tc.tile_pool(name="pp", bufs=1, space="PSUM") as pp,
        ):
            ones = sp.tile([128, 128], fp32, tag="ones")
            nc.gpsimd.memset(ones[:], 1.0)
            bias_sb = sp.tile([1, 1], fp32, tag="bias")
            nc.gpsimd.memset(bias_sb[:], BIAS)

            xt_sb = sp.tile([128, ET, B], e3, tag="xt")
            nc.sync.dma_start(xt_sb[:], xt)
            cs_sb = sp.tile([1, ECN, 128], fp32, tag="cs")
            nc.sync.dma_start(cs_sb[:], cs)

            ps_q = pp.tile([128, FT, B], fp32, tag="psq")
            ps_k = pp.tile([128, FT, B], fp32, tag="psk")
            ps_v = pp.tile([128, FT, B], fp32, tag="psv")
            ps_o = pp.tile([128, ECN, B], fp32, tag="pso")
            ps_pb = pp.tile([128, FT, B], fp32, tag="pspb")
            ps_s = pp.tile([1, 2, HB], fp32, tag="pss")

            # seed the out-proj accumulators with the exact colsum term:
            # ps_o[e', ec, b] = cs[ec*128+e'] (rank-1: colsum x ones).
            # ps_o holds ONE accumulation group: start only on the first
            # matmul (start=True poisons the whole 2KB zero region, so each
            # byte's first write is fresh); stop on the last out-proj matmul.
            for ec in range(ECN):
                nc.tensor.matmul(
                    ps_o[:, ec, :], cs_sb[0:1, ec, :], ones[0:1, :B],
                    start=(ec == 0), stop=False,
                )

            # ---- weight streams ----
            w_sb = {}
            for nm, dram in (("q", wq), ("k", wk), ("v", wv)):
                t_sb = wp.tile([128, ET, F], e3, tag="w" + nm)
                nc.sync.dma_start(t_sb[:], dram)
                w_sb[nm] = t_sb
            wo_sb = wp.tile([128, ECN, FT, 128], e3, tag="wo")
            for ch in range(WCH):
                sl = slice(ch * ECPC, (ch + 1) * ECPC)
                nc.sync.dma_start(wo_sb[:, sl], wo[:, sl])

            # ---- q/k/v projections (W stationary, x moving) ----
            # one accumulation group per psum tile (per weight): start only
            # on the very first matmul, stop on the very last
            for nm, ps in (("q", ps_q), ("k", ps_k), ("v", ps_v)):
                for t in range(ET):
                    for fc in range(FT):
                        nc.tensor.matmul(
                            ps[:, fc, :],
                            w_sb[nm][:, t, fc * 128 : (fc + 1) * 128],
                            xt_sb[:, t, :],
                            start=(t == 0 and fc == 0),
                            stop=(t == ET - 1 and fc == FT - 1),
                        )

            # ---- closed-form attention ----
            q_sb = sp.tile([128, FT, B], fp32, tag="qsb")
            nc.vector.tensor_copy(q_sb[:], ps_q[:])
            qk_sb = sp.tile([128, FT, B], fp32, tag="qksb")
            nc.vector.tensor_tensor(qk_sb[:], q_sb[:], ps_k[:], AL.mult)
            # partition reductions over d: s = ones^T @ (q | q*k)
            nc.tensor.matmul(
                ps_s[:, 0, :], ones[:, 0:1], q_sb[:], start=True, stop=True
            )
            nc.tensor.matmul(
                ps_s[:, 1, :], ones[:, 0:1], qk_sb[:], start=True, stop=True
            )
            s0m = sp.tile([1, HB], fp32, tag="s0m")
            nc.vector.tensor_scalar_mul(s0m[:], ps_s[:, 0, :], SQ)
            tt = sp.tile([1, HB], fp32, tag="tt")
            nc.vector.tensor_tensor(tt[:], ps_s[:, 1, :], s0m[:], AL.subtract)
            p_sb = sp.tile([1, HB], fp32, tag="p")
            nc.scalar.activation(
                p_sb[:], tt[:], mybir.ActivationFunctionType.Sigmoid,
                bias=bias_sb[:], scale=SCALE / (SQ * SQ),
            )
            # broadcast p across partitions (rank-1: ones x p)
            nc.tensor.matmul(
                ps_pb[:], ones[0:1, :], p_sb[:], start=True, stop=True
            )
            t1 = sp.tile([128, FT, B], fp32, tag="t1")
            nc.vector.tensor_scalar(
                t1[:], ps_v[:], SZ / SQ, -SZ, AL.mult, AL.add
            )
            z8 = sp.tile([128, FT, B], e3, tag="z8")
            nc.vector.tensor_tensor(z8[:], t1[:], ps_pb[:], AL.mult)

            # ---- out-proj (W stationary, z moving), chunk-pipelined ----
            o_sb = sp.tile([128, ECN, B], fp32, tag="osb")
            for ch in range(WCH):
                for el in range(ECPC):
                    ec = ch * ECPC + el
                    for ft in range(FT):
                        nc.tensor.matmul(
                            ps_o[:, ec, :],
                            wo_sb[:, ec, ft, :],
                            z8[:, ft, :],
                            start=False,
                            stop=(ec == ECN - 1 and ft == FT - 1),
                        )
            nc.vector.tensor_scalar_mul(o_sb[:], ps_o[:], 1.0 / SO)
            nc.scalar.dma_start(out[:], o_sb[:])

    nc.compile()
    return nc


def _get_program(mode=MODE):
    key = "nc_" + mode
    if key not in _CACHE:
        _CACHE[key] = _build_program()
    return _CACHE[key]


def _shard_inputs(x, Wq, Wk, Wv, Wo):
    import ml_dtypes

    e3 = ml_dtypes.float8_e3m4

    def q8(a):
        return np.clip(a, -15.0, 15.0).astype(e3)

    # xt8[p,t,b] = e3m4(2*x[b, t*128+p]); same for every core
    xt8 = np.ascontiguousarray(
        q8(SX * x.reshape(B, E).T).reshape(ET, 128, B).transpose(1, 0, 2)
    )
    in_maps = []
    for c in range(N_CORES):
        rows = slice(c * F, (c + 1) * F)
        m = {"xt8": xt8}
        for nm, W in (("wq8", Wq), ("wk8", Wk), ("wv8", Wv)):
            # [F,E] slice -> [E,F] -> [128,ET,F] partition-major
            m[nm] = np.ascontiguousarray(
                q8(SW * W[rows, :].T).reshape(ET, 128, F).transpose(1, 0, 2)
            )
        wot = Wo[:, rows].T  # [F, E]
        m["wo8"] = np.ascontiguousarray(
            q8(SW * wot).reshape(FT, 128, ECN, 128).transpose(1, 2, 0, 3)
        )
        m["cs"] = np.ascontiguousarray(
            (SO * wot.astype(np.float32).sum(axis=0)).reshape(1, ECN, 128)
        )
        in_maps.append(m)
    return in_maps


def kernel(x, Wq, Wk, Wv, Wo, _trace=False, **_unused):
    from concourse.bass_utils import run_bass_kernel_spmd

    nc = _get_program()
    in_maps = _shard_inputs(
        np.asarray(x, dtype=np.float32),
        np.asarray(Wq, dtype=np.float32),
        np.asarray(Wk, dtype=np.float32),
        np.asarray(Wv, dtype=np.float32),
        np.asarray(Wo, dtype=np.float32),
    )
    core_ids = list(range(N_CORES))

    def _run(trace):
        return run_bass_kernel_spmd(nc, in_maps, core_ids, trace=trace)

    res = None
    if _trace:
        try:
            res = _run(True)
        except Exception:
            # NTFF profiling hooks unavailable in this environment
            res = None
    if res is None:
        # transient device wedges (NRT_EXEC_UNIT_UNRECOVERABLE) heal after
        # a terminal-side reset; tear down the PJRT client and back off
        # before each retry
        import time as _time

        last = None
        for attempt in range(3):
            try:
                res = _run(False)
                break
            except Exception as e:
                last = e
                try:
                    import jax._src.xla_bridge as _xb

                    _xb._clear_backends()
                except Exception:
                    pass
                _time.sleep(15 * (attempt + 1))
        else:
            raise last
    _CACHE["last_results"] = res
    acc = np.zeros((128, ECN, B), np.float32)
    for r in res.results:
        acc += r["out_p"]
    # out_p[p, ec, b] -> out[b, ec*128+p]
    return np.ascontiguousarray(acc.transpose(2, 1, 0)).reshape(B, 1, E)


# revision 9
# speedup vs baseline: 2.0081x; 1.0481x over previous
"""Trainium2 Bass kernel for nn_MultiHeadAttention_69930657513858.

Single-token (decode) multi-head attention, B=8, E=4096, H=32 heads of
D=128, with a KV cache that is identically ones (length L=4095).

Because the cache is all-ones, attention collapses to a closed form:
  scores = [s0]*L ++ [s1],  s0 = sum_d(q)/sqrt(D), s1 = (q.k)/sqrt(D)
  softmax => p_last = sigmoid(s1 - s0 - ln(L)); cache mass = 1 - p_last
  o = (1 - p_last)*ones + p_last*v = 1 + p_last*(v - 1)
so the kernel is four GEMMs (q,k,v projections + out-proj) plus O(B*H)
scalar work, and the output decomposes as
  out = colsum(Wo) + (p*(v-1)) @ Wo^T
where the colsum term dominates (p is mostly tiny).

The kernel is pure weight streaming (~2 FLOPs/byte), so the only lever
is bytes per weight element.  All four weights ship as fp8 e3m4 (4
mantissa bits), scaled into e3m4's +-15.5 range; the colsum term - the
only place where fp8 rounding of Wo would visibly hurt - is shipped as
an exact fp32 vector (16KB) and seeded into the out-proj PSUM via
rank-1 ones-matmuls, so fp8 error only touches the small correction
term (measured rel err ~4e-3 vs the 2e-2 gate).

Matmuls run W-stationary (weight tile [128,128] stationary, x/z [128,8]
moving), so per-matmul PE time is 8 rows and results come out
transposed [e, b]; the host gather untransposes.  Sharding: tensor
parallel over heads, 4 heads per core; partial out-proj results are
summed on the host (the "all-reduce").

Scale bookkeeping (powers of 2, exact in fp32):
  x*2, W*64 in e3m4  =>  q^,k^,v^ = 128*(q,k,v) in PSUM
  p = sigmoid((s1^ - 128*s0^) * SCALE/128^2 - ln L)
  z8 = e3m4(4*p*(v-1)) = (v^/32 - 4) * p
  psum_out = z8 @ (64*Wo) = 256*corr;  colsum ships pre-scaled *256
  out = psum_out / 256
"""

import math

import numpy as np

B = 8
E = 4096
H = 32
D = 128
L = 4095
N_CORES = 8
HPC = H // N_CORES  # heads per core = 4
F = HPC * D  # per-core head width = 512
ET = E // 128  # contraction tiles for q/k/v = 32
FT = HPC  # contraction tiles for out-proj = 4
ECN = E // 128  # output column chunks for out-proj = 32
HB = HPC * B  # (head, batch) pairs per core = 32
SCALE = 1.0 / math.sqrt(D)
BIAS = -math.log(L)

SX = 2.0  # x pre-scale
SW = 64.0  # weight pre-scale
SZ = 4.0  # z pre-scale
SQ = SX * SW  # q/k/v PSUM scale = 128
SO = SZ * SW  # out-proj PSUM scale = 256

WCH = 8  # wo DMA chunks (4 e-chunks = 256KB each)
ECPC = ECN // WCH  # e-chunks per wo DMA chunk = 4

MODE = "fp8"

_CACHE = {}


def _build_program():
    import concourse.mybir as mybir
    import concourse.tile as tile
    from concourse import bacc

    fp32 = mybir.dt.float32
    e3 = mybir.dt.float8e3
    AL = mybir.AluOpType

    nc = bacc.Bacc("TRN2", target_bir_lowering=False)

    # DRAM layouts are partition-major, prepped on the host:
    #   wq8[p,t,f]     = e3m4(64*Wq[cF+f, t*128+p])      (same wk8, wv8)
    #   wo8[p,ec,ft,e] = e3m4(64*Wo[ec*128+e, cF+ft*128+p])
    #   xt8[p,t,b]     = e3m4(2*x[b, t*128+p])
    #   cs[0,ec,e]     = 256*sum_f Wo[ec*128+e, cF+f]    (fp32, exact)
    wq = nc.dram_tensor("wq8", [128, ET, F], e3, kind="ExternalInput").ap()
    wk = nc.dram_tensor("wk8", [128, ET, F], e3, kind="ExternalInput").ap()
    wv = nc.dram_tensor("wv8", [128, ET, F], e3, kind="ExternalInput").ap()
    wo = nc.dram_tensor("wo8", [128, ECN, FT, 128], e3, kind="ExternalInput").ap()
    xt = nc.dram_tensor("xt8", [128, ET, B], e3, kind="ExternalInput").ap()
    cs = nc.dram_tensor("cs", [1, ECN, 128], fp32, kind="ExternalInput").ap()
    out = nc.dram_tensor("out_p", [128, ECN, B], fp32, kind="ExternalOutput").ap()

    with tile.TileContext(nc) as tc:
        with (
            tc.tile_pool(name="wp", bufs=1) as wp,
            tc.tile_pool(name="sp", bufs=1) as sp,
            tc.tile_pool(name="pp", bufs=1, space="PSUM") as pp,
        ):
            ones = sp.tile([128, 128], fp32, tag="ones")
            nc.gpsimd.memset(ones[:], 1.0)
            bias_sb = sp.tile([1, 1], fp32, tag="bias")
            nc.gpsimd.memset(bias_sb[:], BIAS)

            # wq's DMA is issued first so its HWDGE generation isn't
            # serialized behind the tiny xt/cs transfers (the stream start
            # is generation-gated); xt/cs slot in right after and are done
            # long before the q matmuls need them.
            wq_sb = wp.tile([128, ET, F], e3, tag="wq")
            nc.sync.dma_start(wq_sb[:], wq)
            xt_sb = sp.tile([128, ET, B], e3, tag="xt")
            nc.sync.dma_start(xt_sb[:], xt)
            cs_sb = sp.tile([1, ECN, 128], fp32, tag="cs")
            nc.sync.dma_start(cs_sb[:], cs)

            ps_q = pp.tile([128, FT, B], fp32, tag="psq")
            ps_k = pp.tile([128, FT, B], fp32, tag="psk")
            ps_v = pp.tile([128, FT, B], fp32, tag="psv")
            # out-proj accumulators split in two so the bulk (A) can be
            # drained + stored while the final wo chunk (B) still streams
            ECA = (WCH - 1) * ECPC  # e-chunks in part A = 28
            ps_oa = pp.tile([128, ECA, B], fp32, tag="psoa")
            ps_ob = pp.tile([128, ECN - ECA, B], fp32, tag="psob")
            ps_pb = pp.tile([128, FT, B], fp32, tag="pspb")
            ps_s = pp.tile([1, 2, HB], fp32, tag="pss")

            def ps_o(ec):
                return (ps_oa[:, ec, :], ec == 0) if ec < ECA else (
                    ps_ob[:, ec - ECA, :], ec == ECA
                )

            # seed the out-proj accumulators with the exact colsum term:
            # ps_o[e', ec, b] = cs[ec*128+e'] (rank-1: colsum x ones).
            # each tile holds ONE accumulation group: start only on the first
            # matmul (start=True poisons the whole 2KB zero region, so each
            # byte's first write is fresh); stop on the last out-proj matmul.
            for ec in range(ECN):
                po, st = ps_o(ec)
                nc.tensor.matmul(
                    po, cs_sb[0:1, ec, :], ones[0:1, :B], start=st, stop=False
                )

            # ---- weight streams (wq already issued above) ----
            w_sb = {"q": wq_sb}
            for nm, dram in (("k", wk), ("v", wv)):
                t_sb = wp.tile([128, ET, F], e3, tag="w" + nm)
                nc.sync.dma_start(t_sb[:], dram)
                w_sb[nm] = t_sb
            wo_sb = wp.tile([128, ECN, FT, 128], e3, tag="wo")
            for ch in range(WCH):
                sl = slice(ch * ECPC, (ch + 1) * ECPC)
                nc.sync.dma_start(wo_sb[:, sl], wo[:, sl])

            # ---- q/k/v projections (W stationary, x moving) ----
            # one accumulation group per psum tile (per weight): start only
            # on the very first matmul, stop on the very last
            for nm, ps in (("q", ps_q), ("k", ps_k), ("v", ps_v)):
                for t in range(ET):
                    for fc in range(FT):
                        nc.tensor.matmul(
                            ps[:, fc, :],
                            w_sb[nm][:, t, fc * 128 : (fc + 1) * 128],
                            xt_sb[:, t, :],
                            start=(t == 0 and fc == 0),
                            stop=(t == ET - 1 and fc == FT - 1),
                        )

            # ---- closed-form attention ----
            q_sb = sp.tile([128, FT, B], fp32, tag="qsb")
            nc.vector.tensor_copy(q_sb[:], ps_q[:])
            qk_sb = sp.tile([128, FT, B], fp32, tag="qksb")
            nc.vector.tensor_tensor(qk_sb[:], q_sb[:], ps_k[:], AL.mult)
            # partition reductions over d: s = ones^T @ (q | q*k)
            nc.tensor.matmul(
                ps_s[:, 0, :], ones[:, 0:1], q_sb[:], start=True, stop=True
            )
            nc.tensor.matmul(
                ps_s[:, 1, :], ones[:, 0:1], qk_sb[:], start=True, stop=True
            )
            s0m = sp.tile([1, HB], fp32, tag="s0m")
            nc.vector.tensor_scalar_mul(s0m[:], ps_s[:, 0, :], SQ)
            tt = sp.tile([1, HB], fp32, tag="tt")
            nc.vector.tensor_tensor(tt[:], ps_s[:, 1, :], s0m[:], AL.subtract)
            p_sb = sp.tile([1, HB], fp32, tag="p")
            nc.scalar.activation(
                p_sb[:], tt[:], mybir.ActivationFunctionType.Sigmoid,
                bias=bias_sb[:], scale=SCALE / (SQ * SQ),
            )
            # broadcast p across partitions (rank-1: ones x p)
            nc.tensor.matmul(
                ps_pb[:], ones[0:1, :], p_sb[:], start=True, stop=True
            )
            t1 = sp.tile([128, FT, B], fp32, tag="t1")
            nc.vector.tensor_scalar(
                t1[:], ps_v[:], SZ / SQ, -SZ, AL.mult, AL.add
            )
            z8 = sp.tile([128, FT, B], e3, tag="z8")
            nc.vector.tensor_tensor(z8[:], t1[:], ps_pb[:], AL.mult)

            # ---- out-proj (W stationary, z moving), chunk-pipelined ----
            o_sb = sp.tile([128, ECN, B], fp32, tag="osb")
            for ch in range(WCH):
                for el in range(ECPC):
                    ec = ch * ECPC + el
                    for ft in range(FT):
                        po, _ = ps_o(ec)
                        nc.tensor.matmul(
                            po,
                            wo_sb[:, ec, ft, :],
                            z8[:, ft, :],
                            start=False,
                            stop=(ft == FT - 1 and (ec == ECA - 1 or ec == ECN - 1)),
                        )
                if ch == WCH - 2:
                    # group A complete: drain it under the final wo transfer
                    nc.vector.tensor_scalar_mul(
                        o_sb[:, :ECA], ps_oa[:], 1.0 / SO
                    )
            nc.vector.tensor_scalar_mul(o_sb[:, ECA:], ps_ob[:], 1.0 / SO)
            nc.sync.dma_start(out[:], o_sb[:])

    nc.compile()
    return nc


def _get_program(mode=MODE):
    key = "nc_" + mode
    if key not in _CACHE:
        _CACHE[key] = _build_program()
    return _CACHE[key]


def _shard_inputs(x, Wq, Wk, Wv, Wo):
    import ml_dtypes

    e3 = ml_dtypes.float8_e3m4

    def q8(a):
        return np.clip(a, -15.0, 15.0).astype(e3)

    # xt8[p,t,b] = e3m4(2*x[b, t*128+p]); same for every core
    xt8 = np.ascontiguousarray(
        q8(SX * x.reshape(B, E).T).reshape(ET, 128, B).transpose(1, 0, 2)
    )
    in_maps = []
    for c in range(N_CORES):
        rows = slice(c * F, (c + 1) * F)
        m = {"xt8": xt8}
        for nm, W in (("wq8", Wq), ("wk8", Wk), ("wv8", Wv)):
            # [F,E] slice -> [E,F] -> [128,ET,F] partition-major
            m[nm] = np.ascontiguousarray(
                q8(SW * W[rows, :].T).reshape(ET, 128, F).transpose(1, 0, 2)
            )
        wot = Wo[:, rows].T  # [F, E]
        m["wo8"] = np.ascontiguousarray(
            q8(SW * wot).reshape(FT, 128, ECN, 128).transpose(1, 2, 0, 3)
        )
        m["cs"] = np.ascontiguousarray(
            (SO * wot.astype(np.float32).sum(axis=0)).reshape(1, ECN, 128)
        )
        in_maps.append(m)
    return in_maps


def kernel(x, Wq, Wk, Wv, Wo, _trace=False, **_unused):
    from concourse.bass_utils import run_bass_kernel_spmd

    nc = _get_program()
    in_maps = _shard_inputs(
        np.asarray(x, dtype=np.float32),
        np.asarray(Wq, dtype=np.float32),
        np.asarray(Wk, dtype=np.float32),
        np.asarray(Wv, dtype=np.float32),
        np.asarray(Wo, dtype=np.float32),
    )
    core_ids = list(range(N_CORES))

    def _run(trace):
        return run_bass_kernel_spmd(nc, in_maps, core_ids, trace=trace)

    res = None
    if _trace:
        try:
            res = _run(True)
        except Exception:
            # NTFF profiling hooks unavailable in this environment
            res = None
    if res is None:
        # transient device wedges (NRT_EXEC_UNIT_UNRECOVERABLE) heal after
        # a terminal-side reset; tear down the PJRT client and back off
        # before each retry
        import time as _time

        last = None
        for attempt in range(3):
            try:
                res = _run(False)
                break
            except Exception as e:
                last = e
                try:
                    import jax._src.xla_bridge as _xb

                    _xb._clear_backends()
                except Exception:
                    pass
                _time.sleep(15 * (attempt + 1))
        else:
            raise last
    _CACHE["last_results"] = res
    acc = np.zeros((128, ECN, B), np.float32)
    for r in res.results:
        acc += r["out_p"]
    # out_p[p, ec, b] -> out[b, ec*128+p]
    return np.ascontiguousarray(acc.transpose(2, 1, 0)).reshape(B, 1, E)


# revision 26
# speedup vs baseline: 2.0243x; 1.0081x over previous
"""Trainium2 Bass kernel for nn_MultiHeadAttention_69930657513858.

Single-token (decode) multi-head attention, B=8, E=4096, H=32 heads of
D=128, with a KV cache that is identically ones (length L=4095).

Because the cache is all-ones, attention collapses to a closed form:
  scores = [s0]*L ++ [s1],  s0 = sum_d(q)/sqrt(D), s1 = (q.k)/sqrt(D)
  softmax => p_last = sigmoid(s1 - s0 - ln(L)); cache mass = 1 - p_last
  o = (1 - p_last)*ones + p_last*v = 1 + p_last*(v - 1)
so the kernel is four GEMMs (q,k,v projections + out-proj) plus O(B*H)
scalar work, and the output decomposes as
  out = colsum(Wo) + (p*(v-1)) @ Wo^T
where the colsum term dominates (p is mostly tiny).

The kernel is pure weight streaming (~2 FLOPs/byte), so the only lever
is bytes per weight element.  All four weights ship as fp8 e3m4 (4
mantissa bits), scaled into e3m4's +-15.5 range; the colsum term - the
only place where fp8 rounding of Wo would visibly hurt - is shipped as
an exact fp32 vector (16KB) and seeded into the out-proj PSUM via
rank-1 ones-matmuls, so fp8 error only touches the small correction
term (measured rel err ~4e-3 vs the 2e-2 gate).

Matmuls run W-stationary (weight tile [128,128] stationary, x/z [128,8]
moving), so per-matmul PE time is 8 rows and results come out
transposed [e, b]; the host gather untransposes.  Sharding: tensor
parallel over heads, 4 heads per core; partial out-proj results are
summed on the host (the "all-reduce").

Scale bookkeeping (powers of 2, exact in fp32):
  x*2, W*64 in e3m4  =>  q^,k^,v^ = 128*(q,k,v) in PSUM
  p = sigmoid((s1^ - 128*s0^) * SCALE/128^2 - ln L)
  z8 = e3m4(4*p*(v-1)) = (v^/32 - 4) * p
  psum_out = z8 @ (64*Wo) = 256*corr;  colsum ships pre-scaled *256
  out = psum_out / 256
"""

import math

import numpy as np

B = 8
E = 4096
H = 32
D = 128
L = 4095
N_CORES = 8
HPC = H // N_CORES  # heads per core = 4
F = HPC * D  # per-core head width = 512
ET = E // 128  # contraction tiles for q/k/v = 32
FT = HPC  # contraction tiles for out-proj = 4
ECN = E // 128  # output column chunks for out-proj = 32
HB = HPC * B  # (head, batch) pairs per core = 32
SCALE = 1.0 / math.sqrt(D)
BIAS = -math.log(L)

SX = 2.0  # x pre-scale
SW = 64.0  # weight pre-scale
SZ = 4.0  # z pre-scale
SQ = SX * SW  # q/k/v PSUM scale = 128
SO = SZ * SW  # out-proj PSUM scale = 256

# wo DMA chunk boundaries (in e-chunks): mostly 4-e-chunk (256KB) pieces,
# with a small final chunk so the post-stream matmul+drain wave is short
WO_CHUNKS = [(0, 4), (4, 8), (8, 12), (12, 16), (16, 20), (20, 24),
             (24, 28), (28, 30), (30, 32)]
ECA = 28  # psum group A covers e-chunks [0, ECA); B covers the rest

MODE = "fp8"

_CACHE = {}


def _build_program():
    import concourse.mybir as mybir
    import concourse.tile as tile
    from concourse import bacc

    fp32 = mybir.dt.float32
    e3 = mybir.dt.float8e3
    AL = mybir.AluOpType

    nc = bacc.Bacc("TRN2", target_bir_lowering=False)

    # DRAM layouts are partition-major, prepped on the host:
    #   wq8[p,t,f]     = e3m4(64*Wq[cF+f, t*128+p])      (same wk8, wv8)
    #   wo8[p,ec,ft,e] = e3m4(64*Wo[ec*128+e, cF+ft*128+p])
    #   xt8[p,t,b]     = e3m4(2*x[b, t*128+p])
    #   cs[0,ec,e]     = 256*sum_f Wo[ec*128+e, cF+f]    (fp32, exact)
    # wq ships packed with xt in one DMA (xt alone would pay the sub-512B
    # descriptor penalty): per partition [wq stripe 16KB | xt stripe 256B]
    wqx = nc.dram_tensor(
        "wqx8", [128, ET * F + ET * B], e3, kind="ExternalInput"
    ).ap()
    wk = nc.dram_tensor("wk8", [128, ET, F], e3, kind="ExternalInput").ap()
    wv = nc.dram_tensor("wv8", [128, ET, F], e3, kind="ExternalInput").ap()
    wo = nc.dram_tensor("wo8", [128, ECN, FT, 128], e3, kind="ExternalInput").ap()
    cs = nc.dram_tensor("cs", [1, ECN, 128], fp32, kind="ExternalInput").ap()
    out = nc.dram_tensor("out_p", [128, ECN, B], fp32, kind="ExternalOutput").ap()

    with tile.TileContext(nc) as tc:
        with (
            tc.tile_pool(name="wp", bufs=1) as wp,
            tc.tile_pool(name="sp", bufs=1) as sp,
            tc.tile_pool(name="pp", bufs=1, space="PSUM") as pp,
        ):
            ones = sp.tile([128, 128], fp32, tag="ones")
            nc.gpsimd.memset(ones[:], 1.0)
            bias_sb = sp.tile([1, 1], fp32, tag="bias")
            nc.gpsimd.memset(bias_sb[:], BIAS)
            o_sb = sp.tile([128, ECN, B], fp32, tag="osb")

            # wq(+xt)'s DMA is issued first so its HWDGE generation isn't
            # serialized behind the tiny cs transfer (the stream start is
            # generation-gated); cs slots in right after and is done long
            # before the colsum seed matmuls need it.
            wqx_sb = wp.tile([128, ET * F + ET * B], e3, tag="wqx")
            nc.sync.dma_start(wqx_sb[:], wqx)
            wq_sb = wqx_sb[:, : ET * F].rearrange("p (t f) -> p t f", f=F)
            xt_sb = wqx_sb[:, ET * F :].rearrange("p (t b) -> p t b", b=B)
            cs_sb = sp.tile([1, ECN, 128], fp32, tag="cs")
            nc.sync.dma_start(cs_sb[:], cs)

            ps_q = pp.tile([128, FT, B], fp32, tag="psq")
            ps_k = pp.tile([128, FT, B], fp32, tag="psk")
            ps_v = pp.tile([128, FT, B], fp32, tag="psv")
            # out-proj accumulators split in two so the bulk (A) can be
            # drained + stored while the final wo chunks (B) still stream
            ps_oa = pp.tile([128, ECA, B], fp32, tag="psoa")
            ps_ob = pp.tile([128, ECN - ECA, B], fp32, tag="psob")
            ps_pb = pp.tile([128, FT, B], fp32, tag="pspb")
            ps_s = pp.tile([1, 2, HB], fp32, tag="pss")

            def ps_o(ec):
                return (ps_oa[:, ec, :], ec == 0) if ec < ECA else (
                    ps_ob[:, ec - ECA, :], ec == ECA
                )

            # seed the out-proj accumulators with the exact colsum term:
            # ps_o[e', ec, b] = cs[ec*128+e'] (rank-1: colsum x ones).
            # each tile holds ONE accumulation group: start only on the first
            # matmul (start=True poisons the whole 2KB zero region, so each
            # byte's first write is fresh); stop on the last out-proj matmul.
            for ec in range(ECN):
                po, st = ps_o(ec)
                nc.tensor.matmul(
                    po, cs_sb[0:1, ec, :], ones[0:1, :B], start=st, stop=False
                )

            # ---- weight streams (wq already issued above) ----
            w_sb = {"q": wq_sb}
            for nm, dram in (("k", wk), ("v", wv)):
                t_sb = wp.tile([128, ET, F], e3, tag="w" + nm)
                nc.sync.dma_start(t_sb[:], dram)
                w_sb[nm] = t_sb
            wo_sb = wp.tile([128, ECN, FT, 128], e3, tag="wo")
            for lo, hi in WO_CHUNKS:
                nc.sync.dma_start(wo_sb[:, lo:hi], wo[:, lo:hi])

            # ---- q/k/v projections (W stationary, x moving) ----
            # one accumulation group per psum tile (per weight): start only
            # on the very first matmul, stop on the very last
            for nm, ps in (("q", ps_q), ("k", ps_k), ("v", ps_v)):
                for t in range(ET):
                    for fc in range(FT):
                        nc.tensor.matmul(
                            ps[:, fc, :],
                            w_sb[nm][:, t, fc * 128 : (fc + 1) * 128],
                            xt_sb[:, t, :],
                            start=(t == 0 and fc == 0),
                            stop=(t == ET - 1 and fc == FT - 1),
                        )

            # ---- closed-form attention ----
            q_sb = sp.tile([128, FT, B], fp32, tag="qsb")
            nc.vector.tensor_copy(q_sb[:], ps_q[:])
            qk_sb = sp.tile([128, FT, B], fp32, tag="qksb")
            nc.vector.tensor_tensor(qk_sb[:], q_sb[:], ps_k[:], AL.mult)
            # partition reductions over d: s = ones^T @ (q | q*k)
            nc.tensor.matmul(
                ps_s[:, 0, :], ones[:, 0:1], q_sb[:], start=True, stop=True
            )
            nc.tensor.matmul(
                ps_s[:, 1, :], ones[:, 0:1], qk_sb[:], start=True, stop=True
            )
            s0m = sp.tile([1, HB], fp32, tag="s0m")
            nc.vector.tensor_scalar_mul(s0m[:], ps_s[:, 0, :], SQ)
            tt = sp.tile([1, HB], fp32, tag="tt")
            nc.vector.tensor_tensor(tt[:], ps_s[:, 1, :], s0m[:], AL.subtract)
            p_sb = sp.tile([1, HB], fp32, tag="p")
            nc.scalar.activation(
                p_sb[:], tt[:], mybir.ActivationFunctionType.Sigmoid,
                bias=bias_sb[:], scale=SCALE / (SQ * SQ),
            )
            # broadcast p across partitions (rank-1: ones x p)
            nc.tensor.matmul(
                ps_pb[:], ones[0:1, :], p_sb[:], start=True, stop=True
            )
            t1 = sp.tile([128, FT, B], fp32, tag="t1")
            nc.vector.tensor_scalar(
                t1[:], ps_v[:], SZ / SQ, -SZ, AL.mult, AL.add
            )
            z8 = sp.tile([128, FT, B], e3, tag="z8")
            nc.vector.tensor_tensor(z8[:], t1[:], ps_pb[:], AL.mult)

            # ---- out-proj (W stationary, z moving), chunk-pipelined ----
            for lo, hi in WO_CHUNKS:
                for ec in range(lo, hi):
                    for ft in range(FT):
                        po, _ = ps_o(ec)
                        nc.tensor.matmul(
                            po,
                            wo_sb[:, ec, ft, :],
                            z8[:, ft, :],
                            start=False,
                            stop=(ft == FT - 1 and (ec == ECA - 1 or ec == ECN - 1)),
                        )
                if hi == ECA:
                    # group A complete: drain + store it under the final wo
                    # transfers, so the tail only handles the last ECN-ECA
                    # e-chunks
                    nc.vector.tensor_scalar_mul(
                        o_sb[:, :ECA], ps_oa[:], 1.0 / SO
                    )
                    nc.sync.dma_start(out[:, :ECA], o_sb[:, :ECA])
            nc.vector.tensor_scalar_mul(o_sb[:, ECA:], ps_ob[:], 1.0 / SO)
            nc.sync.dma_start(out[:, ECA:], o_sb[:, ECA:])

    nc.compile()
    return nc


def _get_program(mode=MODE):
    key = "nc_" + mode
    if key not in _CACHE:
        _CACHE[key] = _build_program()
    return _CACHE[key]


def _shard_inputs(x, Wq, Wk, Wv, Wo):
    import ml_dtypes

    e3 = ml_dtypes.float8_e3m4

    def q8(a):
        return np.clip(a, -15.0, 15.0).astype(e3)

    # xt8[p,t,b] = e3m4(2*x[b, t*128+p]); same for every core
    xt8 = (
        q8(SX * x.reshape(B, E).T)
        .reshape(ET, 128, B)
        .transpose(1, 0, 2)
        .reshape(128, ET * B)
    )
    in_maps = []
    for c in range(N_CORES):
        rows = slice(c * F, (c + 1) * F)
        m = {}
        for nm, W in (("wq8", Wq), ("wk8", Wk), ("wv8", Wv)):
            # [F,E] slice -> [E,F] -> [128,ET,F] partition-major
            m[nm] = np.ascontiguousarray(
                q8(SW * W[rows, :].T).reshape(ET, 128, F).transpose(1, 0, 2)
            )
        # pack wq with xt: per partition [wq 16KB | xt 256B]
        m["wqx8"] = np.ascontiguousarray(
            np.concatenate([m.pop("wq8").reshape(128, ET * F), xt8], axis=1)
        )
        wot = Wo[:, rows].T  # [F, E]
        m["wo8"] = np.ascontiguousarray(
            q8(SW * wot).reshape(FT, 128, ECN, 128).transpose(1, 2, 0, 3)
        )
        m["cs"] = np.ascontiguousarray(
            (SO * wot.astype(np.float32).sum(axis=0)).reshape(1, ECN, 128)
        )
        in_maps.append(m)
    return in_maps


def kernel(x, Wq, Wk, Wv, Wo, _trace=False, **_unused):
    from concourse.bass_utils import run_bass_kernel_spmd

    nc = _get_program()
    in_maps = _shard_inputs(
        np.asarray(x, dtype=np.float32),
        np.asarray(Wq, dtype=np.float32),
        np.asarray(Wk, dtype=np.float32),
        np.asarray(Wv, dtype=np.float32),
        np.asarray(Wo, dtype=np.float32),
    )
    core_ids = list(range(N_CORES))

    def _run(trace):
        return run_bass_kernel_spmd(nc, in_maps, core_ids, trace=trace)

    res = None
    if _trace:
        try:
            res = _run(True)
        except Exception:
            # NTFF profiling hooks unavailable in this environment
            res = None
    if res is None:
        # transient device wedges (NRT_EXEC_UNIT_UNRECOVERABLE) heal after
        # a terminal-side reset; tear down the PJRT client and back off
        # before each retry
        import time as _time

        last = None
        for attempt in range(3):
            try:
                res = _run(False)
                break
            except Exception as e:
                last = e
                try:
                    import jax._src.xla_bridge as _xb

                    _xb._clear_backends()
                except Exception:
                    pass
                _time.sleep(15 * (attempt + 1))
        else:
            raise last
    _CACHE["last_results"] = res
    acc = np.zeros((128, ECN, B), np.float32)
    for r in res.results:
        acc += r["out_p"]
    # out_p[p, ec, b] -> out[b, ec*128+p]
    return np.ascontiguousarray(acc.transpose(2, 1, 0)).reshape(B, 1, E)


# revision 31
# speedup vs baseline: 2.0297x; 1.0026x over previous
"""Trainium2 Bass kernel for nn_MultiHeadAttention_69930657513858.

Single-token (decode) multi-head attention, B=8, E=4096, H=32 heads of
D=128, with a KV cache that is identically ones (length L=4095).

Because the cache is all-ones, attention collapses to a closed form:
  scores = [s0]*L ++ [s1],  s0 = sum_d(q)/sqrt(D), s1 = (q.k)/sqrt(D)
  softmax => p_last = sigmoid(s1 - s0 - ln(L)); cache mass = 1 - p_last
  o = (1 - p_last)*ones + p_last*v = 1 + p_last*(v - 1)
so the kernel is four GEMMs (q,k,v projections + out-proj) plus O(B*H)
scalar work, and the output decomposes as
  out = colsum(Wo) + (p*(v-1)) @ Wo^T
where the colsum term dominates (p is mostly tiny).

The kernel is pure weight streaming (~2 FLOPs/byte), so the only lever
is bytes per weight element.  All four weights ship as fp8 e3m4 (4
mantissa bits), scaled into e3m4's +-15.5 range; the colsum term - the
only place where fp8 rounding of Wo would visibly hurt - is shipped as
an exact fp32 vector (16KB) and seeded into the out-proj PSUM via
rank-1 ones-matmuls, so fp8 error only touches the small correction
term (measured rel err ~4e-3 vs the 2e-2 gate).

Matmuls run W-stationary (weight tile [128,128] stationary, x/z [128,8]
moving), so per-matmul PE time is 8 rows and results come out
transposed [e, b]; the host gather untransposes.  Sharding: tensor
parallel over heads, 4 heads per core; partial out-proj results are
summed on the host (the "all-reduce").

Scale bookkeeping (powers of 2, exact in fp32):
  x*2, W*64 in e3m4  =>  q^,k^,v^ = 128*(q,k,v) in PSUM
  p = sigmoid((s1^ - 128*s0^) * SCALE/128^2 - ln L)
  z8 = e3m4(4*p*(v-1)) = (v^/32 - 4) * p
  psum_out = z8 @ (64*Wo) = 256*corr;  colsum ships pre-scaled *256
  out = psum_out / 256
"""

import math

import numpy as np

B = 8
E = 4096
H = 32
D = 128
L = 4095
N_CORES = 8
HPC = H // N_CORES  # heads per core = 4
F = HPC * D  # per-core head width = 512
ET = E // 128  # contraction tiles for q/k/v = 32
FT = HPC  # contraction tiles for out-proj = 4
ECN = E // 128  # output column chunks for out-proj = 32
HB = HPC * B  # (head, batch) pairs per core = 32
SCALE = 1.0 / math.sqrt(D)
BIAS = -math.log(L)

SX = 2.0  # x pre-scale
SW = 64.0  # weight pre-scale
SZ = 4.0  # z pre-scale
SQ = SX * SW  # q/k/v PSUM scale = 128
SO = SZ * SW  # out-proj PSUM scale = 256

# wo DMA chunk boundaries (in e-chunks): mostly 4-e-chunk (256KB) pieces,
# with a small final chunk so the post-stream matmul+drain wave is short
WO_CHUNKS = [(0, 4), (4, 8), (8, 12), (12, 16), (16, 20), (20, 24),
             (24, 28), (28, 30), (30, 32)]
ECA = 24  # psum group A covers e-chunks [0, ECA); B covers the rest

MODE = "fp8"

_CACHE = {}


def _build_program():
    import concourse.mybir as mybir
    import concourse.tile as tile
    from concourse import bacc

    fp32 = mybir.dt.float32
    e3 = mybir.dt.float8e3
    AL = mybir.AluOpType

    nc = bacc.Bacc("TRN2", target_bir_lowering=False)

    # DRAM layouts are partition-major, prepped on the host:
    #   wq8[p,t,f]     = e3m4(64*Wq[cF+f, t*128+p])      (same wk8, wv8)
    #   wo8[p,ec,ft,e] = e3m4(64*Wo[ec*128+e, cF+ft*128+p])
    #   xt8[p,t,b]     = e3m4(2*x[b, t*128+p])
    #   cs[0,ec,e]     = 256*sum_f Wo[ec*128+e, cF+f]    (fp32, exact)
    # wq ships packed with xt in one DMA (xt alone would pay the sub-512B
    # descriptor penalty): per partition [wq stripe 16KB | xt stripe 256B]
    wqx = nc.dram_tensor(
        "wqx8", [128, ET * F + ET * B], e3, kind="ExternalInput"
    ).ap()
    wk = nc.dram_tensor("wk8", [128, ET, F], e3, kind="ExternalInput").ap()
    wv = nc.dram_tensor("wv8", [128, ET, F], e3, kind="ExternalInput").ap()
    wo = nc.dram_tensor("wo8", [128, ECN, FT, 128], e3, kind="ExternalInput").ap()
    cs = nc.dram_tensor("cs", [1, ECN, 128], fp32, kind="ExternalInput").ap()
    out = nc.dram_tensor("out_p", [128, ECN, B], fp32, kind="ExternalOutput").ap()

    with tile.TileContext(nc) as tc:
        with (
            tc.tile_pool(name="wp", bufs=1) as wp,
            tc.tile_pool(name="sp", bufs=1) as sp,
            tc.tile_pool(name="pp", bufs=1, space="PSUM") as pp,
        ):
            # memset on DVE, not gpsimd: the Pool engine is the long pole of
            # the init rendezvous, and putting work there delays the first DMA
            ones = sp.tile([128, 128], fp32, tag="ones")
            nc.vector.memset(ones[:], 1.0)
            bias_sb = sp.tile([1, 1], fp32, tag="bias")
            nc.vector.memset(bias_sb[:], BIAS)
            o_sb = sp.tile([128, ECN, B], fp32, tag="osb")

            # wq(+xt)'s DMA is issued first so its HWDGE generation isn't
            # serialized behind the tiny cs transfer (the stream start is
            # generation-gated); cs slots in right after and is done long
            # before the colsum seed matmuls need it.
            wqx_sb = wp.tile([128, ET * F + ET * B], e3, tag="wqx")
            nc.sync.dma_start(wqx_sb[:], wqx)
            wq_sb = wqx_sb[:, : ET * F].rearrange("p (t f) -> p t f", f=F)
            xt_sb = wqx_sb[:, ET * F :].rearrange("p (t b) -> p t b", b=B)
            cs_sb = sp.tile([1, ECN, 128], fp32, tag="cs")
            nc.sync.dma_start(cs_sb[:], cs)

            ps_q = pp.tile([128, FT, B], fp32, tag="psq")
            ps_k = pp.tile([128, FT, B], fp32, tag="psk")
            ps_v = pp.tile([128, FT, B], fp32, tag="psv")
            # out-proj accumulators split in two so the bulk (A) can be
            # drained + stored while the final wo chunks (B) still stream
            ps_oa = pp.tile([128, ECA, B], fp32, tag="psoa")
            ps_ob = pp.tile([128, ECN - ECA, B], fp32, tag="psob")
            ps_pb = pp.tile([128, FT, B], fp32, tag="pspb")
            ps_s = pp.tile([1, 2, HB], fp32, tag="pss")

            def ps_o(ec):
                return (ps_oa[:, ec, :], ec == 0) if ec < ECA else (
                    ps_ob[:, ec - ECA, :], ec == ECA
                )

            # seed the out-proj accumulators with the exact colsum term:
            # ps_o[e', ec, b] = cs[ec*128+e'] (rank-1: colsum x ones).
            # each tile holds ONE accumulation group: start only on the first
            # matmul (start=True poisons the whole 2KB zero region, so each
            # byte's first write is fresh); stop on the last out-proj matmul.
            for ec in range(ECN):
                po, st = ps_o(ec)
                nc.tensor.matmul(
                    po, cs_sb[0:1, ec, :], ones[0:1, :B], start=st, stop=False
                )

            # ---- weight streams (wq already issued above) ----
            w_sb = {"q": wq_sb}
            for nm, dram in (("k", wk), ("v", wv)):
                t_sb = wp.tile([128, ET, F], e3, tag="w" + nm)
                nc.sync.dma_start(t_sb[:], dram)
                w_sb[nm] = t_sb
            wo_sb = wp.tile([128, ECN, FT, 128], e3, tag="wo")
            for lo, hi in WO_CHUNKS:
                nc.sync.dma_start(wo_sb[:, lo:hi], wo[:, lo:hi])

            # ---- q/k/v projections (W stationary, x moving) ----
            # one accumulation group per psum tile (per weight): start only
            # on the very first matmul, stop on the very last
            for nm, ps in (("q", ps_q), ("k", ps_k), ("v", ps_v)):
                for t in range(ET):
                    for fc in range(FT):
                        nc.tensor.matmul(
                            ps[:, fc, :],
                            w_sb[nm][:, t, fc * 128 : (fc + 1) * 128],
                            xt_sb[:, t, :],
                            start=(t == 0 and fc == 0),
                            stop=(t == ET - 1 and fc == FT - 1),
                        )

            # ---- closed-form attention ----
            q_sb = sp.tile([128, FT, B], fp32, tag="qsb")
            nc.vector.tensor_copy(q_sb[:], ps_q[:])
            qk_sb = sp.tile([128, FT, B], fp32, tag="qksb")
            nc.vector.tensor_tensor(qk_sb[:], q_sb[:], ps_k[:], AL.mult)
            # partition reductions over d: s = ones^T @ (q | q*k)
            nc.tensor.matmul(
                ps_s[:, 0, :], ones[:, 0:1], q_sb[:], start=True, stop=True
            )
            nc.tensor.matmul(
                ps_s[:, 1, :], ones[:, 0:1], qk_sb[:], start=True, stop=True
            )
            s0m = sp.tile([1, HB], fp32, tag="s0m")
            nc.vector.tensor_scalar_mul(s0m[:], ps_s[:, 0, :], SQ)
            tt = sp.tile([1, HB], fp32, tag="tt")
            nc.vector.tensor_tensor(tt[:], ps_s[:, 1, :], s0m[:], AL.subtract)
            p_sb = sp.tile([1, HB], fp32, tag="p")
            nc.scalar.activation(
                p_sb[:], tt[:], mybir.ActivationFunctionType.Sigmoid,
                bias=bias_sb[:], scale=SCALE / (SQ * SQ),
            )
            # broadcast p across partitions (rank-1: ones x p)
            nc.tensor.matmul(
                ps_pb[:], ones[0:1, :], p_sb[:], start=True, stop=True
            )
            t1 = sp.tile([128, FT, B], fp32, tag="t1")
            nc.vector.tensor_scalar(
                t1[:], ps_v[:], SZ / SQ, -SZ, AL.mult, AL.add
            )
            z8 = sp.tile([128, FT, B], e3, tag="z8")
            nc.vector.tensor_tensor(z8[:], t1[:], ps_pb[:], AL.mult)

            # ---- out-proj (W stationary, z moving), chunk-pipelined ----
            for lo, hi in WO_CHUNKS:
                for ec in range(lo, hi):
                    for ft in range(FT):
                        po, _ = ps_o(ec)
                        nc.tensor.matmul(
                            po,
                            wo_sb[:, ec, ft, :],
                            z8[:, ft, :],
                            start=False,
                            stop=(ft == FT - 1 and (ec == ECA - 1 or ec == ECN - 1)),
                        )
                if hi == ECA:
                    # group A complete: drain + store it under the final wo
                    # transfers, so the tail only handles the last ECN-ECA
                    # e-chunks
                    nc.vector.tensor_scalar_mul(
                        o_sb[:, :ECA], ps_oa[:], 1.0 / SO
                    )
                    nc.sync.dma_start(out[:, :ECA], o_sb[:, :ECA])
            nc.vector.tensor_scalar_mul(o_sb[:, ECA:], ps_ob[:], 1.0 / SO)
            nc.sync.dma_start(out[:, ECA:], o_sb[:, ECA:])

    nc.compile()
    return nc


def _get_program(mode=MODE):
    key = "nc_" + mode
    if key not in _CACHE:
        _CACHE[key] = _build_program()
    return _CACHE[key]


def _shard_inputs(x, Wq, Wk, Wv, Wo):
    import ml_dtypes

    e3 = ml_dtypes.float8_e3m4

    def q8(a):
        return np.clip(a, -15.0, 15.0).astype(e3)

    # xt8[p,t,b] = e3m4(2*x[b, t*128+p]); same for every core
    xt8 = (
        q8(SX * x.reshape(B, E).T)
        .reshape(ET, 128, B)
        .transpose(1, 0, 2)
        .reshape(128, ET * B)
    )
    in_maps = []
    for c in range(N_CORES):
        rows = slice(c * F, (c + 1) * F)
        m = {}
        for nm, W in (("wq8", Wq), ("wk8", Wk), ("wv8", Wv)):
            # [F,E] slice -> [E,F] -> [128,ET,F] partition-major
            m[nm] = np.ascontiguousarray(
                q8(SW * W[rows, :].T).reshape(ET, 128, F).transpose(1, 0, 2)
            )
        # pack wq with xt: per partition [wq 16KB | xt 256B]
        m["wqx8"] = np.ascontiguousarray(
            np.concatenate([m.pop("wq8").reshape(128, ET * F), xt8], axis=1)
        )
        wot = Wo[:, rows].T  # [F, E]
        m["wo8"] = np.ascontiguousarray(
            q8(SW * wot).reshape(FT, 128, ECN, 128).transpose(1, 2, 0, 3)
        )
        m["cs"] = np.ascontiguousarray(
            (SO * wot.astype(np.float32).sum(axis=0)).reshape(1, ECN, 128)
        )
        in_maps.append(m)
    return in_maps


def kernel(x, Wq, Wk, Wv, Wo, _trace=False, **_unused):
    from concourse.bass_utils import run_bass_kernel_spmd

    nc = _get_program()
    in_maps = _shard_inputs(
        np.asarray(x, dtype=np.float32),
        np.asarray(Wq, dtype=np.float32),
        np.asarray(Wk, dtype=np.float32),
        np.asarray(Wv, dtype=np.float32),
        np.asarray(Wo, dtype=np.float32),
    )
    core_ids = list(range(N_CORES))

    def _run(trace):
        return run_bass_kernel_spmd(nc, in_maps, core_ids, trace=trace)

    res = None
    if _trace:
        try:
            res = _run(True)
        except Exception:
            # NTFF profiling hooks unavailable in this environment
            res = None
    if res is None:
        # transient device wedges (NRT_EXEC_UNIT_UNRECOVERABLE) heal after
        # a terminal-side reset; tear down the PJRT client and back off
        # before each retry
        import time as _time

        last = None
        for attempt in range(3):
            try:
                res = _run(False)
                break
            except Exception as e:
                last = e
                try:
                    import jax._src.xla_bridge as _xb

                    _xb._clear_backends()
                except Exception:
                    pass
                _time.sleep(15 * (attempt + 1))
        else:
            raise last
    _CACHE["last_results"] = res
    acc = np.zeros((128, ECN, B), np.float32)
    for r in res.results:
        acc += r["out_p"]
    # out_p[p, ec, b] -> out[b, ec*128+p]
    return np.ascontiguousarray(acc.transpose(2, 1, 0)).reshape(B, 1, E)


# revision 32
# speedup vs baseline: 2.0340x; 1.0021x over previous
"""Trainium2 Bass kernel for nn_MultiHeadAttention_69930657513858.

Single-token (decode) multi-head attention, B=8, E=4096, H=32 heads of
D=128, with a KV cache that is identically ones (length L=4095).

Because the cache is all-ones, attention collapses to a closed form:
  scores = [s0]*L ++ [s1],  s0 = sum_d(q)/sqrt(D), s1 = (q.k)/sqrt(D)
  softmax => p_last = sigmoid(s1 - s0 - ln(L)); cache mass = 1 - p_last
  o = (1 - p_last)*ones + p_last*v = 1 + p_last*(v - 1)
so the kernel is four GEMMs (q,k,v projections + out-proj) plus O(B*H)
scalar work, and the output decomposes as
  out = colsum(Wo) + (p*(v-1)) @ Wo^T
where the colsum term dominates (p is mostly tiny).

The kernel is pure weight streaming (~2 FLOPs/byte), so the only lever
is bytes per weight element.  All four weights ship as fp8 e3m4 (4
mantissa bits), scaled into e3m4's +-15.5 range; the colsum term - the
only place where fp8 rounding of Wo would visibly hurt - is shipped as
an exact fp32 vector (16KB) and seeded into the out-proj PSUM via
rank-1 ones-matmuls, so fp8 error only touches the small correction
term (measured rel err ~4e-3 vs the 2e-2 gate).

Matmuls run W-stationary (weight tile [128,128] stationary, x/z [128,8]
moving), so per-matmul PE time is 8 rows and results come out
transposed [e, b]; the host gather untransposes.  Sharding: tensor
parallel over heads, 4 heads per core; partial out-proj results are
summed on the host (the "all-reduce").

Scale bookkeeping (powers of 2, exact in fp32):
  x*2, W*64 in e3m4  =>  q^,k^,v^ = 128*(q,k,v) in PSUM
  p = sigmoid((s1^ - 128*s0^) * SCALE/128^2 - ln L)
  z8 = e3m4(4*p*(v-1)) = (v^/32 - 4) * p
  psum_out = z8 @ (64*Wo) = 256*corr;  colsum ships pre-scaled *256
  out = psum_out / 256
"""

import math

import numpy as np

B = 8
E = 4096
H = 32
D = 128
L = 4095
N_CORES = 8
HPC = H // N_CORES  # heads per core = 4
F = HPC * D  # per-core head width = 512
ET = E // 128  # contraction tiles for q/k/v = 32
FT = HPC  # contraction tiles for out-proj = 4
ECN = E // 128  # output column chunks for out-proj = 32
HB = HPC * B  # (head, batch) pairs per core = 32
SCALE = 1.0 / math.sqrt(D)
BIAS = -math.log(L)

SX = 2.0  # x pre-scale
SW = 64.0  # weight pre-scale
SZ = 4.0  # z pre-scale
SQ = SX * SW  # q/k/v PSUM scale = 128
SO = SZ * SW  # out-proj PSUM scale = 256

# wo DMA chunk boundaries (in e-chunks): mostly 4-e-chunk (256KB) pieces,
# with a small final chunk so the post-stream matmul+drain wave is short
WO_CHUNKS = [(0, 4), (4, 8), (8, 12), (12, 16), (16, 20), (20, 24),
             (24, 28), (28, 30), (30, 31), (31, 32)]
ECA = 24  # psum group A covers e-chunks [0, ECA); B covers the rest

MODE = "fp8"

_CACHE = {}


def _build_program():
    import concourse.mybir as mybir
    import concourse.tile as tile
    from concourse import bacc

    fp32 = mybir.dt.float32
    e3 = mybir.dt.float8e3
    AL = mybir.AluOpType

    nc = bacc.Bacc("TRN2", target_bir_lowering=False)

    # DRAM layouts are partition-major, prepped on the host:
    #   wq8[p,t,f]     = e3m4(64*Wq[cF+f, t*128+p])      (same wk8, wv8)
    #   wo8[p,ec,ft,e] = e3m4(64*Wo[ec*128+e, cF+ft*128+p])
    #   xt8[p,t,b]     = e3m4(2*x[b, t*128+p])
    #   cs[0,ec,e]     = 256*sum_f Wo[ec*128+e, cF+f]    (fp32, exact)
    # wq ships packed with xt in one DMA (xt alone would pay the sub-512B
    # descriptor penalty): per partition [wq stripe 16KB | xt stripe 256B]
    wqx = nc.dram_tensor(
        "wqx8", [128, ET * F + ET * B], e3, kind="ExternalInput"
    ).ap()
    wk = nc.dram_tensor("wk8", [128, ET, F], e3, kind="ExternalInput").ap()
    wv = nc.dram_tensor("wv8", [128, ET, F], e3, kind="ExternalInput").ap()
    wo = nc.dram_tensor("wo8", [128, ECN, FT, 128], e3, kind="ExternalInput").ap()
    cs = nc.dram_tensor("cs", [1, ECN, 128], fp32, kind="ExternalInput").ap()
    out = nc.dram_tensor("out_p", [128, ECN, B], fp32, kind="ExternalOutput").ap()

    with tile.TileContext(nc) as tc:
        with (
            tc.tile_pool(name="wp", bufs=1) as wp,
            tc.tile_pool(name="sp", bufs=1) as sp,
            tc.tile_pool(name="pp", bufs=1, space="PSUM") as pp,
        ):
            # memset on DVE, not gpsimd: the Pool engine is the long pole of
            # the init rendezvous, and putting work there delays the first DMA
            ones = sp.tile([128, 128], fp32, tag="ones")
            nc.vector.memset(ones[:], 1.0)
            bias_sb = sp.tile([1, 1], fp32, tag="bias")
            nc.vector.memset(bias_sb[:], BIAS)
            o_sb = sp.tile([128, ECN, B], fp32, tag="osb")

            # wq(+xt)'s DMA is issued first so its HWDGE generation isn't
            # serialized behind the tiny cs transfer (the stream start is
            # generation-gated); cs slots in right after and is done long
            # before the colsum seed matmuls need it.
            wqx_sb = wp.tile([128, ET * F + ET * B], e3, tag="wqx")
            nc.sync.dma_start(wqx_sb[:], wqx)
            wq_sb = wqx_sb[:, : ET * F].rearrange("p (t f) -> p t f", f=F)
            xt_sb = wqx_sb[:, ET * F :].rearrange("p (t b) -> p t b", b=B)
            cs_sb = sp.tile([1, ECN, 128], fp32, tag="cs")
            nc.sync.dma_start(cs_sb[:], cs)

            ps_q = pp.tile([128, FT, B], fp32, tag="psq")
            ps_k = pp.tile([128, FT, B], fp32, tag="psk")
            ps_v = pp.tile([128, FT, B], fp32, tag="psv")
            # out-proj accumulators split in two so the bulk (A) can be
            # drained + stored while the final wo chunks (B) still stream
            ps_oa = pp.tile([128, ECA, B], fp32, tag="psoa")
            ps_ob = pp.tile([128, ECN - ECA, B], fp32, tag="psob")
            ps_pb = pp.tile([128, FT, B], fp32, tag="pspb")
            ps_s = pp.tile([1, 2, HB], fp32, tag="pss")

            def ps_o(ec):
                return (ps_oa[:, ec, :], ec == 0) if ec < ECA else (
                    ps_ob[:, ec - ECA, :], ec == ECA
                )

            # seed the out-proj accumulators with the exact colsum term:
            # ps_o[e', ec, b] = cs[ec*128+e'] (rank-1: colsum x ones).
            # each tile holds ONE accumulation group: start only on the first
            # matmul (start=True poisons the whole 2KB zero region, so each
            # byte's first write is fresh); stop on the last out-proj matmul.
            for ec in range(ECN):
                po, st = ps_o(ec)
                nc.tensor.matmul(
                    po, cs_sb[0:1, ec, :], ones[0:1, :B], start=st, stop=False
                )

            # ---- weight streams (wq already issued above) ----
            w_sb = {"q": wq_sb}
            for nm, dram in (("k", wk), ("v", wv)):
                t_sb = wp.tile([128, ET, F], e3, tag="w" + nm)
                nc.sync.dma_start(t_sb[:], dram)
                w_sb[nm] = t_sb
            wo_sb = wp.tile([128, ECN, FT, 128], e3, tag="wo")
            for lo, hi in WO_CHUNKS:
                nc.sync.dma_start(wo_sb[:, lo:hi], wo[:, lo:hi])

            # ---- q/k/v projections (W stationary, x moving) ----
            # one accumulation group per psum tile (per weight): start only
            # on the very first matmul, stop on the very last
            for nm, ps in (("q", ps_q), ("k", ps_k), ("v", ps_v)):
                for t in range(ET):
                    for fc in range(FT):
                        nc.tensor.matmul(
                            ps[:, fc, :],
                            w_sb[nm][:, t, fc * 128 : (fc + 1) * 128],
                            xt_sb[:, t, :],
                            start=(t == 0 and fc == 0),
                            stop=(t == ET - 1 and fc == FT - 1),
                        )

            # ---- closed-form attention ----
            q_sb = sp.tile([128, FT, B], fp32, tag="qsb")
            nc.vector.tensor_copy(q_sb[:], ps_q[:])
            qk_sb = sp.tile([128, FT, B], fp32, tag="qksb")
            nc.vector.tensor_tensor(qk_sb[:], q_sb[:], ps_k[:], AL.mult)
            # partition reductions over d: s = ones^T @ (q | q*k)
            nc.tensor.matmul(
                ps_s[:, 0, :], ones[:, 0:1], q_sb[:], start=True, stop=True
            )
            nc.tensor.matmul(
                ps_s[:, 1, :], ones[:, 0:1], qk_sb[:], start=True, stop=True
            )
            s0m = sp.tile([1, HB], fp32, tag="s0m")
            nc.vector.tensor_scalar_mul(s0m[:], ps_s[:, 0, :], SQ)
            tt = sp.tile([1, HB], fp32, tag="tt")
            nc.vector.tensor_tensor(tt[:], ps_s[:, 1, :], s0m[:], AL.subtract)
            p_sb = sp.tile([1, HB], fp32, tag="p")
            nc.scalar.activation(
                p_sb[:], tt[:], mybir.ActivationFunctionType.Sigmoid,
                bias=bias_sb[:], scale=SCALE / (SQ * SQ),
            )
            # broadcast p across partitions (rank-1: ones x p)
            nc.tensor.matmul(
                ps_pb[:], ones[0:1, :], p_sb[:], start=True, stop=True
            )
            t1 = sp.tile([128, FT, B], fp32, tag="t1")
            nc.vector.tensor_scalar(
                t1[:], ps_v[:], SZ / SQ, -SZ, AL.mult, AL.add
            )
            z8 = sp.tile([128, FT, B], e3, tag="z8")
            nc.vector.tensor_tensor(z8[:], t1[:], ps_pb[:], AL.mult)

            # ---- out-proj (W stationary, z moving), chunk-pipelined ----
            for lo, hi in WO_CHUNKS:
                for ec in range(lo, hi):
                    for ft in range(FT):
                        po, _ = ps_o(ec)
                        nc.tensor.matmul(
                            po,
                            wo_sb[:, ec, ft, :],
                            z8[:, ft, :],
                            start=False,
                            stop=(ft == FT - 1 and (ec == ECA - 1 or ec == ECN - 1)),
                        )
                if hi == ECA:
                    # group A complete: drain + store it under the final wo
                    # transfers, so the tail only handles the last ECN-ECA
                    # e-chunks
                    nc.vector.tensor_scalar_mul(
                        o_sb[:, :ECA], ps_oa[:], 1.0 / SO
                    )
                    nc.sync.dma_start(out[:, :ECA], o_sb[:, :ECA])
            nc.vector.tensor_scalar_mul(o_sb[:, ECA:], ps_ob[:], 1.0 / SO)
            nc.sync.dma_start(out[:, ECA:], o_sb[:, ECA:])

    nc.compile()
    return nc


def _get_program(mode=MODE):
    key = "nc_" + mode
    if key not in _CACHE:
        _CACHE[key] = _build_program()
    return _CACHE[key]


def _shard_inputs(x, Wq, Wk, Wv, Wo):
    import ml_dtypes

    e3 = ml_dtypes.float8_e3m4

    def q8(a):
        return np.clip(a, -15.0, 15.0).astype(e3)

    # xt8[p,t,b] = e3m4(2*x[b, t*128+p]); same for every core
    xt8 = (
        q8(SX * x.reshape(B, E).T)
        .reshape(ET, 128, B)
        .transpose(1, 0, 2)
        .reshape(128, ET * B)
    )
    in_maps = []
    for c in range(N_CORES):
        rows = slice(c * F, (c + 1) * F)
        m = {}
        for nm, W in (("wq8", Wq), ("wk8", Wk), ("wv8", Wv)):
            # [F,E] slice -> [E,F] -> [128,ET,F] partition-major
            m[nm] = np.ascontiguousarray(
                q8(SW * W[rows, :].T).reshape(ET, 128, F).transpose(1, 0, 2)
            )
        # pack wq with xt: per partition [wq 16KB | xt 256B]
        m["wqx8"] = np.ascontiguousarray(
            np.concatenate([m.pop("wq8").reshape(128, ET * F), xt8], axis=1)
        )
        wot = Wo[:, rows].T  # [F, E]
        m["wo8"] = np.ascontiguousarray(
            q8(SW * wot).reshape(FT, 128, ECN, 128).transpose(1, 2, 0, 3)
        )
        m["cs"] = np.ascontiguousarray(
            (SO * wot.astype(np.float32).sum(axis=0)).reshape(1, ECN, 128)
        )
        in_maps.append(m)
    return in_maps


def kernel(x, Wq, Wk, Wv, Wo, _trace=False, **_unused):
    from concourse.bass_utils import run_bass_kernel_spmd

    nc = _get_program()
    in_maps = _shard_inputs(
        np.asarray(x, dtype=np.float32),
        np.asarray(Wq, dtype=np.float32),
        np.asarray(Wk, dtype=np.float32),
        np.asarray(Wv, dtype=np.float32),
        np.asarray(Wo, dtype=np.float32),
    )
    core_ids = list(range(N_CORES))

    def _run(trace):
        return run_bass_kernel_spmd(nc, in_maps, core_ids, trace=trace)

    res = None
    if _trace:
        try:
            res = _run(True)
        except Exception:
            # NTFF profiling hooks unavailable in this environment
            res = None
    if res is None:
        # transient device wedges (NRT_EXEC_UNIT_UNRECOVERABLE) heal after
        # a terminal-side reset; tear down the PJRT client and back off
        # before each retry
        import time as _time

        last = None
        for attempt in range(3):
            try:
                res = _run(False)
                break
            except Exception as e:
                last = e
                try:
                    import jax._src.xla_bridge as _xb

                    _xb._clear_backends()
                except Exception:
                    pass
                _time.sleep(15 * (attempt + 1))
        else:
            raise last
    _CACHE["last_results"] = res
    acc = np.zeros((128, ECN, B), np.float32)
    for r in res.results:
        acc += r["out_p"]
    # out_p[p, ec, b] -> out[b, ec*128+p]
    return np.ascontiguousarray(acc.transpose(2, 1, 0)).reshape(B, 1, E)


# revision 36
# speedup vs baseline: 2.0402x; 1.0031x over previous
"""Trainium2 Bass kernel for nn_MultiHeadAttention_69930657513858.

Single-token (decode) multi-head attention, B=8, E=4096, H=32 heads of
D=128, with a KV cache that is identically ones (length L=4095).

Because the cache is all-ones, attention collapses to a closed form:
  scores = [s0]*L ++ [s1],  s0 = sum_d(q)/sqrt(D), s1 = (q.k)/sqrt(D)
  softmax => p_last = sigmoid(s1 - s0 - ln(L)); cache mass = 1 - p_last
  o = (1 - p_last)*ones + p_last*v = 1 + p_last*(v - 1)
so the kernel is four GEMMs (q,k,v projections + out-proj) plus O(B*H)
scalar work, and the output decomposes as
  out = colsum(Wo) + (p*(v-1)) @ Wo^T
where the colsum term dominates (p is mostly tiny).

The kernel is pure weight streaming (~2 FLOPs/byte), so the only lever
is bytes per weight element.  All four weights ship as fp8 e3m4 (4
mantissa bits), scaled into e3m4's +-15.5 range; the colsum term - the
only place where fp8 rounding of Wo would visibly hurt - is shipped as
an exact fp32 vector (16KB) and seeded into the out-proj PSUM via
rank-1 ones-matmuls, so fp8 error only touches the small correction
term (measured rel err ~4e-3 vs the 2e-2 gate).

Matmuls run W-stationary (weight tile [128,128] stationary, x/z [128,8]
moving), so per-matmul PE time is 8 rows and results come out
transposed [e, b]; the host gather untransposes.  Sharding: tensor
parallel over heads, 4 heads per core; partial out-proj results are
summed on the host (the "all-reduce").

Scale bookkeeping (powers of 2, exact in fp32):
  x*2, W*64 in e3m4  =>  q^,k^,v^ = 128*(q,k,v) in PSUM
  p = sigmoid((s1^ - 128*s0^) * SCALE/128^2 - ln L)
  z8 = e3m4(4*p*(v-1)) = (v^/32 - 4) * p
  psum_out = z8 @ (64*Wo) = 256*corr;  colsum ships pre-scaled *256
  out = psum_out / 256
"""

import math

import numpy as np

B = 8
E = 4096
H = 32
D = 128
L = 4095
N_CORES = 8
HPC = H // N_CORES  # heads per core = 4
F = HPC * D  # per-core head width = 512
ET = E // 128  # contraction tiles for q/k/v = 32
FT = HPC  # contraction tiles for out-proj = 4
ECN = E // 128  # output column chunks for out-proj = 32
HB = HPC * B  # (head, batch) pairs per core = 32
SCALE = 1.0 / math.sqrt(D)
BIAS = -math.log(L)

SX = 2.0  # x pre-scale
SW = 64.0  # weight pre-scale
SZ = 4.0  # z pre-scale
SQ = SX * SW  # q/k/v PSUM scale = 128
SO = SZ * SW  # out-proj PSUM scale = 256

# wo DMA chunk boundaries (in e-chunks): mostly 4-e-chunk (256KB) pieces,
# with a small final chunk so the post-stream matmul+drain wave is short
WO_CHUNKS = [(0, 4), (4, 8), (8, 12), (12, 16), (16, 20), (20, 24),
             (24, 28), (28, 30), (30, 31), (31, 32)]
ECA = 24  # psum group A covers e-chunks [0, ECA); B covers the rest

MODE = "fp8"

_CACHE = {}


def _build_program():
    import concourse.mybir as mybir
    import concourse.tile as tile
    from concourse import bacc

    fp32 = mybir.dt.float32
    e3 = mybir.dt.float8e3
    AL = mybir.AluOpType

    nc = bacc.Bacc("TRN2", target_bir_lowering=False)

    # DRAM layouts are partition-major, prepped on the host:
    #   wq8[p,t,f]     = e3m4(64*Wq[cF+f, t*128+p])      (same wk8, wv8)
    #   wo8[p,ec,ft,e] = e3m4(64*Wo[ec*128+e, cF+ft*128+p])
    #   xt8[p,t,b]     = e3m4(2*x[b, t*128+p])
    #   cs[0,ec,e]     = 256*sum_f Wo[ec*128+e, cF+f]    (fp32, exact)
    # wq ships packed with xt in one DMA (xt alone would pay the sub-512B
    # descriptor penalty): per partition [wq stripe 16KB | xt stripe 256B]
    wqx = nc.dram_tensor(
        "wqx8", [128, ET * F + ET * B], e3, kind="ExternalInput"
    ).ap()
    wk = nc.dram_tensor("wk8", [128, ET, F], e3, kind="ExternalInput").ap()
    wv = nc.dram_tensor("wv8", [128, ET, F], e3, kind="ExternalInput").ap()
    wo = nc.dram_tensor("wo8", [128, ECN, FT, 128], e3, kind="ExternalInput").ap()
    cs = nc.dram_tensor("cs", [1, ECN, 128], fp32, kind="ExternalInput").ap()
    out = nc.dram_tensor("out_p", [128, ECA, B], fp32, kind="ExternalOutput").ap()
    # the tail-critical B-region store ships as bf16: its per-partition run
    # is under 512B either way (2x descriptor penalty), so halving the bytes
    # halves the store time; bf16 on 1/8th of one core's partial adds ~0.07%
    # error
    bf16 = mybir.dt.bfloat16
    out_b = nc.dram_tensor(
        "out_b", [128, ECN - ECA, B], bf16, kind="ExternalOutput"
    ).ap()

    with tile.TileContext(nc) as tc:
        with (
            tc.tile_pool(name="wp", bufs=1) as wp,
            tc.tile_pool(name="sp", bufs=1) as sp,
            tc.tile_pool(name="pp", bufs=1, space="PSUM") as pp,
        ):
            # memset on DVE, not gpsimd: the Pool engine is the long pole of
            # the init rendezvous, and putting work there delays the first DMA
            ones = sp.tile([128, 128], fp32, tag="ones")
            nc.vector.memset(ones[:], 1.0)
            bias_sb = sp.tile([1, 1], fp32, tag="bias")
            nc.vector.memset(bias_sb[:], BIAS)
            o_sb = sp.tile([128, ECA, B], fp32, tag="osb")
            ob_sb = sp.tile([128, ECN - ECA, B], bf16, tag="obsb")

            # wq(+xt)'s DMA is issued first so its HWDGE generation isn't
            # serialized behind the tiny cs transfer (the stream start is
            # generation-gated); cs slots in right after and is done long
            # before the colsum seed matmuls need it.
            wqx_sb = wp.tile([128, ET * F + ET * B], e3, tag="wqx")
            nc.sync.dma_start(wqx_sb[:], wqx)
            wq_sb = wqx_sb[:, : ET * F].rearrange("p (t f) -> p t f", f=F)
            xt_sb = wqx_sb[:, ET * F :].rearrange("p (t b) -> p t b", b=B)
            cs_sb = sp.tile([1, ECN, 128], fp32, tag="cs")
            nc.sync.dma_start(cs_sb[:], cs)

            ps_q = pp.tile([128, FT, B], fp32, tag="psq")
            ps_k = pp.tile([128, FT, B], fp32, tag="psk")
            ps_v = pp.tile([128, FT, B], fp32, tag="psv")
            # out-proj accumulators split in two so the bulk (A) can be
            # drained + stored while the final wo chunks (B) still stream
            ps_oa = pp.tile([128, ECA, B], fp32, tag="psoa")
            ps_ob = pp.tile([128, ECN - ECA, B], fp32, tag="psob")
            ps_pb = pp.tile([128, FT, B], fp32, tag="pspb")
            ps_s = pp.tile([1, 2, HB], fp32, tag="pss")

            def ps_o(ec):
                return (ps_oa[:, ec, :], ec == 0) if ec < ECA else (
                    ps_ob[:, ec - ECA, :], ec == ECA
                )

            # seed the out-proj accumulators with the exact colsum term:
            # ps_o[e', ec, b] = cs[ec*128+e'] (rank-1: colsum x ones).
            # each tile holds ONE accumulation group: start only on the first
            # matmul (start=True poisons the whole 2KB zero region, so each
            # byte's first write is fresh); stop on the last out-proj matmul.
            for ec in range(ECN):
                po, st = ps_o(ec)
                nc.tensor.matmul(
                    po, cs_sb[0:1, ec, :], ones[0:1, :B], start=st, stop=False
                )

            # ---- weight streams (wq already issued above) ----
            w_sb = {"q": wq_sb}
            for nm, dram in (("k", wk), ("v", wv)):
                t_sb = wp.tile([128, ET, F], e3, tag="w" + nm)
                nc.sync.dma_start(t_sb[:], dram)
                w_sb[nm] = t_sb
            wo_sb = wp.tile([128, ECN, FT, 128], e3, tag="wo")
            for lo, hi in WO_CHUNKS:
                nc.sync.dma_start(wo_sb[:, lo:hi], wo[:, lo:hi])

            # ---- q/k/v projections (W stationary, x moving) ----
            # one accumulation group per psum tile (per weight): start only
            # on the very first matmul, stop on the very last
            for nm, ps in (("q", ps_q), ("k", ps_k), ("v", ps_v)):
                for t in range(ET):
                    for fc in range(FT):
                        nc.tensor.matmul(
                            ps[:, fc, :],
                            w_sb[nm][:, t, fc * 128 : (fc + 1) * 128],
                            xt_sb[:, t, :],
                            start=(t == 0 and fc == 0),
                            stop=(t == ET - 1 and fc == FT - 1),
                        )

            # ---- closed-form attention ----
            q_sb = sp.tile([128, FT, B], fp32, tag="qsb")
            nc.vector.tensor_copy(q_sb[:], ps_q[:])
            qk_sb = sp.tile([128, FT, B], fp32, tag="qksb")
            nc.vector.tensor_tensor(qk_sb[:], q_sb[:], ps_k[:], AL.mult)
            # partition reductions over d: s = ones^T @ (q | q*k)
            nc.tensor.matmul(
                ps_s[:, 0, :], ones[:, 0:1], q_sb[:], start=True, stop=True
            )
            nc.tensor.matmul(
                ps_s[:, 1, :], ones[:, 0:1], qk_sb[:], start=True, stop=True
            )
            s0m = sp.tile([1, HB], fp32, tag="s0m")
            nc.vector.tensor_scalar_mul(s0m[:], ps_s[:, 0, :], SQ)
            tt = sp.tile([1, HB], fp32, tag="tt")
            nc.vector.tensor_tensor(tt[:], ps_s[:, 1, :], s0m[:], AL.subtract)
            p_sb = sp.tile([1, HB], fp32, tag="p")
            nc.scalar.activation(
                p_sb[:], tt[:], mybir.ActivationFunctionType.Sigmoid,
                bias=bias_sb[:], scale=SCALE / (SQ * SQ),
            )
            # broadcast p across partitions (rank-1: ones x p)
            nc.tensor.matmul(
                ps_pb[:], ones[0:1, :], p_sb[:], start=True, stop=True
            )
            t1 = sp.tile([128, FT, B], fp32, tag="t1")
            nc.vector.tensor_scalar(
                t1[:], ps_v[:], SZ / SQ, -SZ, AL.mult, AL.add
            )
            z8 = sp.tile([128, FT, B], e3, tag="z8")
            nc.vector.tensor_tensor(z8[:], t1[:], ps_pb[:], AL.mult)

            # ---- out-proj (W stationary, z moving), chunk-pipelined ----
            for lo, hi in WO_CHUNKS:
                for ec in range(lo, hi):
                    for ft in range(FT):
                        po, _ = ps_o(ec)
                        nc.tensor.matmul(
                            po,
                            wo_sb[:, ec, ft, :],
                            z8[:, ft, :],
                            start=False,
                            stop=(ft == FT - 1 and (ec == ECA - 1 or ec == ECN - 1)),
                        )
                if hi == ECA:
                    # group A complete: drain + store it under the final wo
                    # transfers, so the tail only handles the last ECN-ECA
                    # e-chunks
                    nc.vector.tensor_scalar_mul(o_sb[:], ps_oa[:], 1.0 / SO)
                    nc.sync.dma_start(out[:], o_sb[:])
            nc.vector.tensor_scalar_mul(ob_sb[:], ps_ob[:], 1.0 / SO)
            nc.sync.dma_start(out_b[:], ob_sb[:])

    nc.compile()
    return nc


def _get_program(mode=MODE):
    key = "nc_" + mode
    if key not in _CACHE:
        _CACHE[key] = _build_program()
    return _CACHE[key]


def _shard_inputs(x, Wq, Wk, Wv, Wo):
    import ml_dtypes

    e3 = ml_dtypes.float8_e3m4

    def q8(a):
        return np.clip(a, -15.0, 15.0).astype(e3)

    # xt8[p,t,b] = e3m4(2*x[b, t*128+p]); same for every core
    xt8 = (
        q8(SX * x.reshape(B, E).T)
        .reshape(ET, 128, B)
        .transpose(1, 0, 2)
        .reshape(128, ET * B)
    )
    in_maps = []
    for c in range(N_CORES):
        rows = slice(c * F, (c + 1) * F)
        m = {}
        for nm, W in (("wq8", Wq), ("wk8", Wk), ("wv8", Wv)):
            # [F,E] slice -> [E,F] -> [128,ET,F] partition-major
            m[nm] = np.ascontiguousarray(
                q8(SW * W[rows, :].T).reshape(ET, 128, F).transpose(1, 0, 2)
            )
        # pack wq with xt: per partition [wq 16KB | xt 256B]
        m["wqx8"] = np.ascontiguousarray(
            np.concatenate([m.pop("wq8").reshape(128, ET * F), xt8], axis=1)
        )
        wot = Wo[:, rows].T  # [F, E]
        m["wo8"] = np.ascontiguousarray(
            q8(SW * wot).reshape(FT, 128, ECN, 128).transpose(1, 2, 0, 3)
        )
        m["cs"] = np.ascontiguousarray(
            (SO * wot.astype(np.float32).sum(axis=0)).reshape(1, ECN, 128)
        )
        in_maps.append(m)
    return in_maps


def kernel(x, Wq, Wk, Wv, Wo, _trace=False, **_unused):
    from concourse.bass_utils import run_bass_kernel_spmd

    nc = _get_program()
    in_maps = _shard_inputs(
        np.asarray(x, dtype=np.float32),
        np.asarray(Wq, dtype=np.float32),
        np.asarray(Wk, dtype=np.float32),
        np.asarray(Wv, dtype=np.float32),
        np.asarray(Wo, dtype=np.float32),
    )
    core_ids = list(range(N_CORES))

    def _run(trace):
        return run_bass_kernel_spmd(nc, in_maps, core_ids, trace=trace)

    res = None
    if _trace:
        try:
            res = _run(True)
        except Exception:
            # NTFF profiling hooks unavailable in this environment
            res = None
    if res is None:
        # transient device wedges (NRT_EXEC_UNIT_UNRECOVERABLE) heal after
        # a terminal-side reset; tear down the PJRT client and back off
        # before each retry
        import time as _time

        last = None
        for attempt in range(3):
            try:
                res = _run(False)
                break
            except Exception as e:
                last = e
                try:
                    import jax._src.xla_bridge as _xb

                    _xb._clear_backends()
                except Exception:
                    pass
                _time.sleep(15 * (attempt + 1))
        else:
            raise last
    _CACHE["last_results"] = res
    acc = np.zeros((128, ECN, B), np.float32)
    for r in res.results:
        acc[:, :ECA] += r["out_p"]
        acc[:, ECA:] += r["out_b"].astype(np.float32)
    # [p, ec, b] -> out[b, ec*128+p]
    return np.ascontiguousarray(acc.transpose(2, 1, 0)).reshape(B, 1, E)


# revision 39
# speedup vs baseline: 2.0442x; 1.0020x over previous
"""Trainium2 Bass kernel for nn_MultiHeadAttention_69930657513858.

Single-token (decode) multi-head attention, B=8, E=4096, H=32 heads of
D=128, with a KV cache that is identically ones (length L=4095).

Because the cache is all-ones, attention collapses to a closed form:
  scores = [s0]*L ++ [s1],  s0 = sum_d(q)/sqrt(D), s1 = (q.k)/sqrt(D)
  softmax => p_last = sigmoid(s1 - s0 - ln(L)); cache mass = 1 - p_last
  o = (1 - p_last)*ones + p_last*v = 1 + p_last*(v - 1)
so the kernel is four GEMMs (q,k,v projections + out-proj) plus O(B*H)
scalar work, and the output decomposes as
  out = colsum(Wo) + (p*(v-1)) @ Wo^T
where the colsum term dominates (p is mostly tiny).

The kernel is pure weight streaming (~2 FLOPs/byte), so the only lever
is bytes per weight element.  All four weights ship as fp8 e3m4 (4
mantissa bits), scaled into e3m4's +-15.5 range; the colsum term - the
only place where fp8 rounding of Wo would visibly hurt - is shipped as
an exact fp32 vector (16KB) and seeded into the out-proj PSUM via
rank-1 ones-matmuls, so fp8 error only touches the small correction
term (measured rel err ~4e-3 vs the 2e-2 gate).

Matmuls run W-stationary (weight tile [128,128] stationary, x/z [128,8]
moving), so per-matmul PE time is 8 rows and results come out
transposed [e, b]; the host gather untransposes.  Sharding: tensor
parallel over heads, 4 heads per core; partial out-proj results are
summed on the host (the "all-reduce").

Scale bookkeeping (powers of 2, exact in fp32):
  x*2, W*64 in e3m4  =>  q^,k^,v^ = 128*(q,k,v) in PSUM
  p = sigmoid((s1^ - 128*s0^) * SCALE/128^2 - ln L)
  z8 = e3m4(4*p*(v-1)) = (v^/32 - 4) * p
  psum_out = z8 @ (64*Wo) = 256*corr;  colsum ships pre-scaled *256
  out = psum_out / 256
"""

import math

import numpy as np

B = 8
E = 4096
H = 32
D = 128
L = 4095
N_CORES = 8
HPC = H // N_CORES  # heads per core = 4
F = HPC * D  # per-core head width = 512
ET = E // 128  # contraction tiles for q/k/v = 32
FT = HPC  # contraction tiles for out-proj = 4
ECN = E // 128  # output column chunks for out-proj = 32
HB = HPC * B  # (head, batch) pairs per core = 32
SCALE = 1.0 / math.sqrt(D)
BIAS = -math.log(L)

SX = 2.0  # x pre-scale
SW = 64.0  # weight pre-scale
SZ = 4.0  # z pre-scale
SQ = SX * SW  # q/k/v PSUM scale = 128
SO = SZ * SW  # out-proj PSUM scale = 256

# wo DMA chunk boundaries (in e-chunks): mostly 4-e-chunk (256KB) pieces,
# with a small final chunk so the post-stream matmul+drain wave is short
WO_CHUNKS = [(0, 4), (4, 8), (8, 12), (12, 16), (16, 20), (20, 24),
             (24, 28), (28, 30), (30, 31), (31, 32)]
ECA = 24  # psum group A covers e-chunks [0, ECA); B covers the rest

MODE = "fp8"

_CACHE = {}


def _build_program():
    import concourse.mybir as mybir
    import concourse.tile as tile
    from concourse import bacc

    fp32 = mybir.dt.float32
    e3 = mybir.dt.float8e3
    AL = mybir.AluOpType

    nc = bacc.Bacc("TRN2", target_bir_lowering=False)

    # DRAM layouts are partition-major, prepped on the host:
    #   wq8[p,t,f]     = e3m4(64*Wq[cF+f, t*128+p])      (same wk8, wv8)
    #   wo8[p,ec,ft,e] = e3m4(64*Wo[ec*128+e, cF+ft*128+p])
    #   xt8[p,t,b]     = e3m4(2*x[b, t*128+p])
    #   cs[0,ec,e]     = 256*sum_f Wo[ec*128+e, cF+f]    (fp32, exact)
    # wq ships packed with xt in one DMA (xt alone would pay the sub-512B
    # descriptor penalty): per partition [wq stripe 16KB | xt stripe 256B]
    wqx = nc.dram_tensor(
        "wqx8", [128, ET * F + ET * B], e3, kind="ExternalInput"
    ).ap()
    wk = nc.dram_tensor("wk8", [128, ET, F], e3, kind="ExternalInput").ap()
    wv = nc.dram_tensor("wv8", [128, ET, F], e3, kind="ExternalInput").ap()
    wo = nc.dram_tensor("wo8", [128, ECN, FT, 128], e3, kind="ExternalInput").ap()
    cs = nc.dram_tensor("cs", [1, ECN, 128], fp32, kind="ExternalInput").ap()
    out = nc.dram_tensor("out_p", [128, ECA, B], fp32, kind="ExternalOutput").ap()
    # the tail-critical B-region store ships as bf16: its per-partition run
    # is under 512B either way (2x descriptor penalty), so halving the bytes
    # halves the store time; bf16 on 1/8th of one core's partial adds ~0.07%
    # error
    bf16 = mybir.dt.bfloat16
    out_b = nc.dram_tensor(
        "out_b", [128, ECN - ECA, B], bf16, kind="ExternalOutput"
    ).ap()

    with tile.TileContext(nc) as tc:
        with (
            tc.tile_pool(name="wp", bufs=1) as wp,
            tc.tile_pool(name="sp", bufs=1) as sp,
            tc.tile_pool(name="pp", bufs=1, space="PSUM") as pp,
        ):
            # memset on DVE, not gpsimd: the Pool engine is the long pole of
            # the init rendezvous, and putting work there delays the first DMA
            ones = sp.tile([128, 128], fp32, tag="ones")
            nc.vector.memset(ones[:], 1.0)
            bias_sb = sp.tile([1, 1], fp32, tag="bias")
            nc.vector.memset(bias_sb[:], BIAS)
            o_sb = sp.tile([128, ECA, B], fp32, tag="osb")
            ob_sb = sp.tile([128, ECN - ECA, B], bf16, tag="obsb")

            # wq(+xt)'s DMA is issued first so its HWDGE generation isn't
            # serialized behind the tiny cs transfer (the stream start is
            # generation-gated); cs slots in right after and is done long
            # before the colsum seed matmuls need it.
            wqx_sb = wp.tile([128, ET * F + ET * B], e3, tag="wqx")
            nc.sync.dma_start(wqx_sb[:], wqx)
            wq_sb = wqx_sb[:, : ET * F].rearrange("p (t f) -> p t f", f=F)
            xt_sb = wqx_sb[:, ET * F :].rearrange("p (t b) -> p t b", b=B)
            cs_sb = sp.tile([1, ECN, 128], fp32, tag="cs")
            nc.sync.dma_start(cs_sb[:], cs)

            ps_q = pp.tile([128, FT, B], fp32, tag="psq")
            ps_k = pp.tile([128, FT, B], fp32, tag="psk")
            ps_v = pp.tile([128, FT, B], fp32, tag="psv")
            # out-proj accumulators split in two so the bulk (A) can be
            # drained + stored while the final wo chunks (B) still stream
            ps_oa = pp.tile([128, ECA, B], fp32, tag="psoa")
            # B splits again: ob1 (ec ECA..30) drains at the penultimate
            # chunk's arrival; ob2 (ec 31 alone) is all the final drain covers
            ps_ob1 = pp.tile([128, ECN - ECA - 1, B], fp32, tag="psob1")
            ps_ob2 = pp.tile([128, 1, B], fp32, tag="psob2")
            ps_pb = pp.tile([128, FT, B], fp32, tag="pspb")
            ps_s = pp.tile([1, 2, HB], fp32, tag="pss")

            def ps_o(ec):
                if ec < ECA:
                    return ps_oa[:, ec, :], ec == 0
                if ec < ECN - 1:
                    return ps_ob1[:, ec - ECA, :], ec == ECA
                return ps_ob2[:, 0, :], True

            # seed the out-proj accumulators with the exact colsum term:
            # ps_o[e', ec, b] = cs[ec*128+e'] (rank-1: colsum x ones).
            # each tile holds ONE accumulation group: start only on the first
            # matmul (start=True poisons the whole 2KB zero region, so each
            # byte's first write is fresh); stop on the last out-proj matmul.
            for ec in range(ECN):
                po, st = ps_o(ec)
                nc.tensor.matmul(
                    po, cs_sb[0:1, ec, :], ones[0:1, :B], start=st, stop=False
                )

            # ---- weight streams (wq already issued above) ----
            w_sb = {"q": wq_sb}
            for nm, dram in (("k", wk), ("v", wv)):
                t_sb = wp.tile([128, ET, F], e3, tag="w" + nm)
                nc.sync.dma_start(t_sb[:], dram)
                w_sb[nm] = t_sb
            wo_sb = wp.tile([128, ECN, FT, 128], e3, tag="wo")
            for lo, hi in WO_CHUNKS:
                nc.sync.dma_start(wo_sb[:, lo:hi], wo[:, lo:hi])

            # ---- q/k/v projections (W stationary, x moving) ----
            # one accumulation group per psum tile (per weight): start only
            # on the very first matmul, stop on the very last
            for nm, ps in (("q", ps_q), ("k", ps_k), ("v", ps_v)):
                for t in range(ET):
                    for fc in range(FT):
                        nc.tensor.matmul(
                            ps[:, fc, :],
                            w_sb[nm][:, t, fc * 128 : (fc + 1) * 128],
                            xt_sb[:, t, :],
                            start=(t == 0 and fc == 0),
                            stop=(t == ET - 1 and fc == FT - 1),
                        )

            # ---- closed-form attention ----
            q_sb = sp.tile([128, FT, B], fp32, tag="qsb")
            nc.vector.tensor_copy(q_sb[:], ps_q[:])
            qk_sb = sp.tile([128, FT, B], fp32, tag="qksb")
            nc.vector.tensor_tensor(qk_sb[:], q_sb[:], ps_k[:], AL.mult)
            # partition reductions over d: s = ones^T @ (q | q*k)
            nc.tensor.matmul(
                ps_s[:, 0, :], ones[:, 0:1], q_sb[:], start=True, stop=True
            )
            nc.tensor.matmul(
                ps_s[:, 1, :], ones[:, 0:1], qk_sb[:], start=True, stop=True
            )
            s0m = sp.tile([1, HB], fp32, tag="s0m")
            nc.vector.tensor_scalar_mul(s0m[:], ps_s[:, 0, :], SQ)
            tt = sp.tile([1, HB], fp32, tag="tt")
            nc.vector.tensor_tensor(tt[:], ps_s[:, 1, :], s0m[:], AL.subtract)
            p_sb = sp.tile([1, HB], fp32, tag="p")
            nc.scalar.activation(
                p_sb[:], tt[:], mybir.ActivationFunctionType.Sigmoid,
                bias=bias_sb[:], scale=SCALE / (SQ * SQ),
            )
            # broadcast p across partitions (rank-1: ones x p)
            nc.tensor.matmul(
                ps_pb[:], ones[0:1, :], p_sb[:], start=True, stop=True
            )
            t1 = sp.tile([128, FT, B], fp32, tag="t1")
            nc.vector.tensor_scalar(
                t1[:], ps_v[:], SZ / SQ, -SZ, AL.mult, AL.add
            )
            z8 = sp.tile([128, FT, B], e3, tag="z8")
            nc.vector.tensor_tensor(z8[:], t1[:], ps_pb[:], AL.mult)

            # ---- out-proj (W stationary, z moving), chunk-pipelined ----
            for lo, hi in WO_CHUNKS:
                for ec in range(lo, hi):
                    for ft in range(FT):
                        po, _ = ps_o(ec)
                        nc.tensor.matmul(
                            po,
                            wo_sb[:, ec, ft, :],
                            z8[:, ft, :],
                            start=False,
                            stop=(
                                ft == FT - 1
                                and ec in (ECA - 1, ECN - 2, ECN - 1)
                            ),
                        )
                if hi == ECN - 1:
                    nc.vector.tensor_scalar_mul(
                        ob_sb[:, : ECN - ECA - 1], ps_ob1[:], 1.0 / SO
                    )
                if hi == ECA:
                    # group A complete: drain + store it under the final wo
                    # transfers, so the tail only handles the last ECN-ECA
                    # e-chunks
                    nc.vector.tensor_scalar_mul(o_sb[:], ps_oa[:], 1.0 / SO)
                    nc.sync.dma_start(out[:], o_sb[:])
            nc.vector.tensor_scalar_mul(
                ob_sb[:, ECN - ECA - 1 :], ps_ob2[:], 1.0 / SO
            )
            nc.sync.dma_start(out_b[:], ob_sb[:])

    nc.compile()
    return nc


def _get_program(mode=MODE):
    key = "nc_" + mode
    if key not in _CACHE:
        _CACHE[key] = _build_program()
    return _CACHE[key]


def _shard_inputs(x, Wq, Wk, Wv, Wo):
    import ml_dtypes

    e3 = ml_dtypes.float8_e3m4

    def q8(a):
        return np.clip(a, -15.0, 15.0).astype(e3)

    # xt8[p,t,b] = e3m4(2*x[b, t*128+p]); same for every core
    xt8 = (
        q8(SX * x.reshape(B, E).T)
        .reshape(ET, 128, B)
        .transpose(1, 0, 2)
        .reshape(128, ET * B)
    )
    in_maps = []
    for c in range(N_CORES):
        rows = slice(c * F, (c + 1) * F)
        m = {}
        for nm, W in (("wq8", Wq), ("wk8", Wk), ("wv8", Wv)):
            # [F,E] slice -> [E,F] -> [128,ET,F] partition-major
            m[nm] = np.ascontiguousarray(
                q8(SW * W[rows, :].T).reshape(ET, 128, F).transpose(1, 0, 2)
            )
        # pack wq with xt: per partition [wq 16KB | xt 256B]
        m["wqx8"] = np.ascontiguousarray(
            np.concatenate([m.pop("wq8").reshape(128, ET * F), xt8], axis=1)
        )
        wot = Wo[:, rows].T  # [F, E]
        m["wo8"] = np.ascontiguousarray(
            q8(SW * wot).reshape(FT, 128, ECN, 128).transpose(1, 2, 0, 3)
        )
        m["cs"] = np.ascontiguousarray(
            (SO * wot.astype(np.float32).sum(axis=0)).reshape(1, ECN, 128)
        )
        in_maps.append(m)
    return in_maps


def kernel(x, Wq, Wk, Wv, Wo, _trace=False, **_unused):
    from concourse.bass_utils import run_bass_kernel_spmd

    nc = _get_program()
    in_maps = _shard_inputs(
        np.asarray(x, dtype=np.float32),
        np.asarray(Wq, dtype=np.float32),
        np.asarray(Wk, dtype=np.float32),
        np.asarray(Wv, dtype=np.float32),
        np.asarray(Wo, dtype=np.float32),
    )
    core_ids = list(range(N_CORES))

    def _run(trace):
        return run_bass_kernel_spmd(nc, in_maps, core_ids, trace=trace)

    res = None
    if _trace:
        try:
            res = _run(True)
        except Exception:
            # NTFF profiling hooks unavailable in this environment
            res = None
    if res is None:
        # transient device wedges (NRT_EXEC_UNIT_UNRECOVERABLE) heal after
        # a terminal-side reset; tear down the PJRT client and back off
        # before each retry
        import time as _time

        last = None
        for attempt in range(3):
            try:
                res = _run(False)
                break
            except Exception as e:
                last = e
                try:
                    import jax._src.xla_bridge as _xb

                    _xb._clear_backends()
                except Exception:
                    pass
                _time.sleep(15 * (attempt + 1))
        else:
            raise last
    _CACHE["last_results"] = res
    acc = np.zeros((128, ECN, B), np.float32)
    for r in res.results:
        acc[:, :ECA] += r["out_p"]
        acc[:, ECA:] += r["out_b"].astype(np.float32)
    # [p, ec, b] -> out[b, ec*128+p]
    return np.ascontiguousarray(acc.transpose(2, 1, 0)).reshape(B, 1, E)
